# revision 22
# baseline (speedup 1.0000x reference)
"""AdMSoftmaxLoss on 8 TRN2 NeuronCores (Bass/Tile).

Math (matches the reference exactly):
    xn    = x / ||x||_row
    wf    = xn @ W.T                      [N, C]
    t_i   = wf[i, y_i]
    num_i = s*(t_i - m)
    den_i = exp(num_i) + sum_j exp(s*wf_ij) - exp(s*t_i)
    loss  = -mean(num_i - log(den_i))

Distribution: vocab/tensor parallel. W's class dim is sharded 8 ways;
each core computes its slice of the logits as an fp8e4 DoubleRow matmul.
x is row-normalized on the host, so the exp scale is a compile-time
constant and PSUM drains have no data dependency on a norms pass.

v2 structure (from trace analysis of v1):
- PSUM per (ni, m) tile is split into two pool tiles, psS (<=3 banks,
  ScalarE table-Exp drain w/ accum_out row-sums) and psD (1 bank, DVE
  Schraudolph exp2 drain). With one shared tile the framework chained
  DVE's psum read behind ScalarE's accumulator drain, making the drain
  chain ~3.2us > the 2.1us GPIO-throttled matmul fill and costing ~1us
  every other tile.
- The target-dot products are single gpsimd scalar_tensor_tensor
  instructions (mult, mult, accum_out=tz). v1 put the dot row-sum
  reduces on DVE, where the first one head-of-line blocked the in-order
  DVE queue for 38us waiting on the SWDGE gathers.
- Class dim padded 12500 -> 12544 (44 pad cols, v1 used 300).
- First W tile is 1024 cols so the matmul ramp starts earlier; xq is
  split across the scalar+vector DMA queues.
- Phase A AllReduce (class tiles 0-4 + target partials) overlaps tiles
  5-6; its epilogue math is precomputed mid-body so the exposed tail is
  only: last drain -> AllReduce B -> den add/log -> output.
"""

import math

import ml_dtypes
import numpy as np

import concourse.bacc as bacc
import concourse.bass_isa as bass_isa
import concourse.bass as bass
import concourse.mybir as mybir
import concourse.tile as tile
from concourse.bass_utils import run_bass_kernel_spmd

N, D, C, CORES = 4096, 512, 100000, 8
CSH = C // CORES
S_SCALE, MARGIN = 30.0, 0.4

F32 = mybir.dt.float32
BF16 = mybir.dt.bfloat16
I32 = mybir.dt.int32
FP8 = mybir.dt.float8e4
AF = mybir.ActivationFunctionType
OP = mybir.AluOpType
AX = mybir.AxisListType
ReduceOp = bass_isa.ReduceOp

XSCALE, WSCALE = 360.0, 256.0
KEXP = S_SCALE / (XSCALE * WSCALE)          # exp scale on raw psum values
A_SCH = (1 << 23) * math.log2(math.e) * KEXP  # Schraudolph multiplier
B_SCH = 1.0648707e9                           # tuned magic constant

# Per-core class dim padded 12500 -> 12544 with zero-weight columns.
# NTL entries: (start, width, ScalarE share width). ScalarE share is a
# multiple of 512 so the psS/psD pool tiles stay PSUM-bank aligned.
CSHP = 12544
# 12544 = 7 uniform tiles of 1792: ScalarE drains 1280 cols/tile (80%
# of the 1.84us GPIO-throttled fill), DVE 512 (74%) -- both engines
# keep slack so psum WAR never stalls the matmul stream.
NTL = [(i * 1792, 1792, 1280) for i in range(7)]
assert NTL[-1][0] + NTL[-1][1] == CSHP
NNT = len(NTL)
MMW = 512  # matmul moving width (ISA limit: 512 psum cols)
# pad cols 12500..12544 sit in the last tile's DVE share (global
# [12032,12544)): each contributes the exactly computable Schraudolph
# constant for input 0.
_C0 = float(np.int32(int(np.float32(0.0) * np.float32(A_SCH)
                         + np.float32(B_SCH))).view(np.float32))
C_PAD_TOTAL = CORES * 44 * _C0

import os
K_NODOT = bool(int(os.environ.get("K_NODOT", "0")))
K_NOGATHER = bool(int(os.environ.get("K_NOGATHER", "0")))


def build(n=N, d=D, csh=CSH, cores=CORES):
    mt, kt2 = n // 128, d // 256
    nc = bacc.Bacc("TRN2", target_bir_lowering=False, debug=False, num_devices=cores)

    # host-packed DoubleRow layouts: [ki, k2, ko, *] with d = k2*256 + ko*128 + ki
    xq_d = nc.dram_tensor("xq", [128, kt2, 2, n], FP8, kind="ExternalInput")
    xn_d = nc.dram_tensor("xn", [128, mt, d], BF16, kind="ExternalInput")
    wq_d = nc.dram_tensor("wq", [128, kt2, 2, CSHP], FP8, kind="ExternalInput")
    # one extra all-zeros row: out-of-shard labels gather it (no memsets)
    wn_d = nc.dram_tensor("wn", [csh + 1, d], BF16, kind="ExternalInput")
    off_d = nc.dram_tensor("off", [128, mt], I32, kind="ExternalInput")
    out_d = nc.dram_tensor("out", [1, 1], F32, kind="ExternalOutput")
    ccA_in = nc.dram_tensor("ccA_in", [128, 2 * mt], F32)
    ccA_out = nc.dram_tensor("ccA_out", [128, 2 * mt], F32, addr_space="Shared")
    ccB_in = nc.dram_tensor("ccB_in", [128, mt], F32)
    ccB_out = nc.dram_tensor("ccB_out", [128, mt], F32, addr_space="Shared")
    barz_d = nc.dram_tensor("barz", [128, 1], F32, kind="ExternalInput")
    bar_in_d = nc.dram_tensor("bar_in", [128, 1], F32)
    bar_out_d = nc.dram_tensor("bar_out", [128, 1], F32, addr_space="Shared")

    with tile.TileContext(nc) as tc:
        with (
            tc.tile_pool(name="const", bufs=1) as cpool,
            tc.tile_pool(name="wstream", bufs=4) as wpool,
            tc.tile_pool(name="escr", bufs=3) as epool,
            tc.tile_pool(name="iscr", bufs=3) as ipool,
            tc.tile_pool(name="dscr", bufs=2) as dpool,
            tc.tile_pool(name="psum", bufs=2, space="PSUM") as ppool,
        ):
            # start-of-kernel barrier: a [128,1] AllReduce of host-zeros
            # whose result becomes the drain ACT bias. Every core's drain
            # pipeline (and via psum WAR, its matmul stream) is gated on
            # global start alignment, under the shadow of the initial
            # DMA loads. Aligned cores -> the phase A/B collectives wait
            # ~2.5us instead of 20-100us on a straggler. The input is an
            # ExternalInput (ready at t=0, no init chain) and the
            # collective is emitted at priority 0 so the tile scheduler
            # keeps it at the head of the gpsimd queue.
            with tc.high_priority():
                # collectives can't read IO tensors: bounce the host
                # zeros through an internal DRAM scratch first
                nc.scalar.dma_start(bar_in_d[:, :], barz_d[:, :])
                nc.gpsimd.collective_compute(
                    "AllReduce", OP.add, replica_groups=[list(range(cores))],
                    ins=[bar_in_d.ap().opt()], outs=[bar_out_d.ap().opt()])

            # stationary x^T (fp8 DoubleRow) on the scalar DMA queue
            xts = cpool.tile([128, kt2, 2, n], FP8, tag="xts")
            nc.scalar.dma_start(xts[:, :, :, :], xq_d[:, :, :, :])
            bias_sb = cpool.tile([128, 1], F32, tag="bias_sb")
            nc.scalar.dma_start(bias_sb[:, :], bar_out_d[:, :])
            xr_all = cpool.tile([128, mt, d], BF16, tag="xr_all")
            # W stream + xn ride the sync queue; xn after the first two W
            # tiles so the matmul ramp isn't starved.
            wts = []
            for ni, (n0, nw, _) in enumerate(NTL):
                wt = wpool.tile([128, kt2, 2, 2048], FP8, tag="wt", name="wt")
                nc.sync.dma_start(wt[:, :, :, :nw], wq_d[:, :, :, n0 : n0 + nw])
                wts.append(wt)
                if ni == 1:
                    nc.sync.dma_start(xr_all[:, :, :], xn_d[:, :, :])

            # target-row gathers + dot products, all on the gpsimd queue:
            # 32 indirect SWDGE gathers, then 32 single-instruction
            # scalar_tensor_tensor dots (out is a scratch dummy; the row
            # dot rides accum_out). Nothing here touches DVE/ScalarE.
            off_sb = cpool.tile([128, mt], I32, tag="off")
            nc.gpsimd.dma_start(off_sb[:, :], off_d[:, :])
            wy_all = cpool.tile([128, mt, d], BF16, tag="wy_all")
            if not K_NOGATHER:
                for m in range(mt):
                    nc.gpsimd.indirect_dma_start(
                        out=wy_all[:, m, :], out_offset=None, in_=wn_d[:, :],
                        in_offset=bass.IndirectOffsetOnAxis(
                            ap=off_sb[:, m : m + 1], axis=0),
                        bounds_check=csh, oob_is_err=False)

            # gpsimd has no free-axis reduce, so the row dots are a
            # batched binary tree fold (bf16 products -> f32 halves ->
            # f32 folds), never touching the DVE/ScalarE queues.
            tz = cpool.tile([128, mt], F32, tag="tz")
            if K_NODOT:
                nc.vector.memset(tz[:, :], 0.0)
            else:
                pr_all = cpool.tile([128, mt, d], BF16, tag="pr_all")
                for m in range(mt):
                    nc.gpsimd.tensor_tensor(
                        out=pr_all[:, m, :], in0=xr_all[:, m, :],
                        in1=wy_all[:, m, :], op=OP.mult)
                prh = cpool.tile([128, mt, d // 2], F32, tag="prh")
                nc.gpsimd.tensor_tensor(
                    out=prh[:, :, :], in0=pr_all[:, :, 0 : d // 2],
                    in1=pr_all[:, :, d // 2 : d], op=OP.add)
                w = d // 4
                while w >= 1:
                    nc.gpsimd.tensor_tensor(
                        out=prh[:, :, 0:w], in0=prh[:, :, 0:w],
                        in1=prh[:, :, w : 2 * w], op=OP.add)
                    w //= 2
                nc.gpsimd.tensor_copy(out=tz[:, :], in_=prh[:, :, 0])

            # per-engine accumulators (separate tiles so ScalarE accum
            # writes and DVE reduce writes never WAW-serialize)
            accS = cpool.tile([128, mt, NNT], F32, tag="accS")
            accD = cpool.tile([128, mt, NNT], F32, tag="accD")

            lnc = cpool.tile([128, 1], F32, tag="lnc")
            nc.vector.memset(
                lnc[:, :], float(math.log(1.0 - math.exp(-S_SCALE * MARGIN))))
            csm = cpool.tile([128, 1], F32, tag="csm")
            nc.vector.memset(csm[:, :], float(S_SCALE * MARGIN))

            # main loop: fp8 DoubleRow matmuls into per-engine psum tiles
            # (psS: ScalarE table Exp w/ free accum row-sum; psD: DVE
            # Schraudolph exp2 via i32 affine + bitcast + reduce).
            NA = 5  # ntiles in collective phase A (classes [0, 9216))
            rr = cpool.tile([128, 2 * mt], F32, tag="rr")
            Bs = cpool.tile([128, 2], F32, tag="Bs")
            denA = cpool.tile([128, mt], F32, tag="denA")
            for ni, (n0, nw, scw) in enumerate(NTL):
                wt = wts[ni]
                dvw = nw - scw
                # chunk so no matmul write crosses a PSUM bank boundary
                # (psS/psD tiles are bank-aligned; scw itself need not
                # be a multiple of 512)
                nsub = [(j0, min(MMW, scw - j0)) for j0 in range(0, scw, MMW)]
                nsub += [(scw + j0, min(MMW, dvw - j0))
                         for j0 in range(0, dvw, MMW)]
                for m in range(mt):
                    psS = ppool.tile([128, 1536], F32, tag="psS", name="psS")
                    psD = ppool.tile([128, 512], F32, tag="psD", name="psD")
                    for k2 in range(kt2):
                        for j0, jw in nsub:
                            if j0 < scw:
                                dst = psS[:, j0 : j0 + jw]
                            else:
                                dst = psD[:, j0 - scw : j0 - scw + jw]
                            nc.tensor.matmul(
                                out=dst,
                                lhsT=xts[:, k2, :, m * 128 : (m + 1) * 128],
                                rhs=wt[:, k2, :, j0 : j0 + jw],
                                start=(k2 == 0), stop=(k2 == kt2 - 1),
                                perf_mode=mybir.MatmulPerfMode.DoubleRow)
                    # ScalarE share: exp -> bf16 with free row-sum accum
                    # (bias is the all-zeros barrier result: math no-op,
                    # gates the drain stream on the start barrier)
                    et = epool.tile([128, 1536], BF16, tag="et", name="et")
                    nc.scalar.activation(
                        et[:, :scw], psS[:, :scw], AF.Exp,
                        bias=bias_sb[:, 0:1], scale=KEXP,
                        accum_out=accS[:, m, ni : ni + 1])
                    # DVE share: Schraudolph exp2 (i32 convert + bitcast)
                    ti = ipool.tile([128, 512], I32, tag="ti", name="ti")
                    nc.vector.tensor_scalar(
                        out=ti[:, :dvw], in0=psD[:, :dvw],
                        scalar1=float(A_SCH), scalar2=float(B_SCH),
                        op0=OP.mult, op1=OP.add)
                    nc.vector.tensor_reduce(
                        out=accD[:, m, ni : ni + 1],
                        in_=ti[:, :dvw].bitcast(F32), axis=AX.X, op=OP.add)
                if ni == NA - 1:
                    # phase A: expsum partials for classes [0, 9216) + all
                    # target-logit partials; AllReduce overlaps ni 5-6.
                    ccsbA = cpool.tile([128, 2 * mt], F32, tag="ccsbA")
                    tmpA = cpool.tile([128, mt], F32, tag="tmpA")
                    nc.vector.tensor_reduce(
                        out=ccsbA[:, 0:mt], in_=accS[:, :, 0:NA],
                        axis=AX.X, op=OP.add)
                    nc.vector.tensor_reduce(
                        out=tmpA[:, :], in_=accD[:, :, 0:NA], axis=AX.X, op=OP.add)
                    nc.vector.tensor_tensor(
                        out=ccsbA[:, 0:mt], in0=ccsbA[:, 0:mt], in1=tmpA[:, :],
                        op=OP.add)
                    nc.vector.tensor_copy(out=ccsbA[:, mt : 2 * mt], in_=tz[:, :])
                    nc.sync.dma_start(ccA_in[:, :], ccsbA[:, :])
                    nc.gpsimd.collective_compute(
                        "AllReduce", OP.add,
                        replica_groups=[list(range(cores))],
                        ins=[ccA_in.ap().opt()], outs=[ccA_out.ap().opt()])
                if ni == NA:
                    # phase-A result readback (sync queue; waits on the
                    # collective, but nothing downstream consumes it
                    # until the tail, so no engine queue blocks on it)
                    nc.sync.dma_start(rr[:, :], ccA_out[:, :])

            # phase B: expsum partials for classes [9216, 12544)
            ccsbB = cpool.tile([128, mt], F32, tag="ccsbB")
            tmpB = cpool.tile([128, mt], F32, tag="tmpB")
            nc.vector.tensor_reduce(
                out=ccsbB[:, 0:mt], in_=accS[:, :, NA:NNT], axis=AX.X, op=OP.add)
            nc.vector.tensor_reduce(
                out=tmpB[:, :], in_=accD[:, :, NA:NNT], axis=AX.X, op=OP.add)
            nc.vector.tensor_tensor(
                out=ccsbB[:, 0:mt], in0=ccsbB[:, 0:mt], in1=tmpB[:, :], op=OP.add)
            nc.sync.dma_start(ccB_in[:, :], ccsbB[:, :])
            nc.gpsimd.collective_compute(
                "AllReduce", OP.add, replica_groups=[list(range(cores))],
                ins=[ccB_in.ap().opt()], outs=[ccB_out.ap().opt()])

            # phase-A epilogue (emitted at the tail so it can't
            # head-of-line block the Scalar/Vector queues behind the
            # collective): B = s*t_tot; Bs0 = sum(B);
            # Es = exp(B + ln(1-e^{-sm})); denA = expsumA - Es - pads
            B = cpool.tile([128, mt], F32, tag="B")
            nc.vector.tensor_scalar(
                out=B[:, :], in0=rr[:, mt : 2 * mt], scalar1=S_SCALE,
                scalar2=None, op0=OP.mult, op1=OP.add, accum_out=Bs[:, 0:1])
            Es = cpool.tile([128, mt], F32, tag="Es")
            nc.scalar.activation(
                Es[:, :], B[:, :], AF.Exp, bias=lnc[:, :1], scale=1.0)
            nc.vector.tensor_tensor(
                out=denA[:, :], in0=rr[:, 0:mt], in1=Es[:, :], op=OP.subtract)
            nc.vector.tensor_scalar(
                out=denA[:, :], in0=denA[:, :], scalar1=float(C_PAD_TOTAL),
                scalar2=None, op0=OP.subtract)

            rrB = cpool.tile([128, mt], F32, tag="rrB")
            nc.sync.dma_start(rrB[:, :], ccB_out[:, :])

            # exposed tail: den = denA + expsumB; loss = s*m - (sum(B) -
            # sum(ln den))/n
            den = cpool.tile([128, mt], F32, tag="den")
            nc.vector.tensor_tensor(
                out=den[:, :], in0=denA[:, :], in1=rrB[:, :], op=OP.add)
            lden = cpool.tile([128, mt], F32, tag="lden")
            nc.scalar.activation(
                lden[:, :], den[:, :], AF.Ln, accum_out=Bs[:, 1:2])
            diff = cpool.tile([128, 1], F32, tag="diff")
            nc.vector.tensor_tensor(
                out=diff[:, :], in0=Bs[:, 0:1], in1=Bs[:, 1:2], op=OP.subtract)
            zs = cpool.tile([128, 1], F32, tag="zs")
            nc.gpsimd.partition_all_reduce(zs[:, :], diff[:, :], 128, ReduceOp.add)
            res = cpool.tile([1, 1], F32, tag="res")
            nc.scalar.activation(
                res[:, :], zs[:1, :], AF.Identity,
                bias=csm[:1, :], scale=-1.0 / n)
            nc.sync.dma_start(out_d[:, :], res[:, :])
    nc.compile()
    return nc


def shard_inputs(x, labels, W, n=N, d=D, csh=CSH, cores=CORES):
    x32 = np.ascontiguousarray(np.asarray(x), dtype=np.float32)
    xn = x32 / np.sqrt((x32 * x32).sum(axis=1, keepdims=True))
    # [ki, k2, ko, n] fp8 DoubleRow layout: d = k2*256 + ko*128 + ki
    xq = (xn.T * XSCALE).reshape(2, 2, 128, n).transpose(2, 0, 1, 3)
    xq = np.ascontiguousarray(np.clip(xq, -240, 240)).astype(ml_dtypes.float8_e4m3)
    # [p, m, d] bf16 rows
    xnb = np.ascontiguousarray(
        xn.reshape(n // 128, 128, d).transpose(1, 0, 2)).astype(ml_dtypes.bfloat16)
    lab = np.asarray(labels).astype(np.int64).reshape(n)
    in_maps = []
    for r in range(cores):
        Wc = np.ascontiguousarray(np.asarray(W)[r * csh : (r + 1) * csh], dtype=np.float32)
        Wp = np.zeros((CSHP, d), np.float32)
        Wp[:csh] = Wc
        wq = (Wp.T * WSCALE).reshape(2, 2, 128, CSHP).transpose(2, 0, 1, 3)
        wq = np.ascontiguousarray(np.clip(wq, -240, 240)).astype(ml_dtypes.float8_e4m3)
        loc = lab - r * csh
        off = np.where((loc >= 0) & (loc < csh), loc, csh).astype(np.int32)
        off = np.ascontiguousarray(off.reshape(n // 128, 128).T)
        wn = np.vstack([Wc, np.zeros((1, d), np.float32)]).astype(ml_dtypes.bfloat16)
        in_maps.append({
            "xq": xq, "xn": xnb, "wq": wq,
            "wn": np.ascontiguousarray(wn), "off": off,
            "barz": np.zeros((128, 1), np.float32),
        })
    return in_maps


_CACHE = {}


def kernel(x, labels, W, **run_kwargs):
    if "nc" not in _CACHE:
        _CACHE["nc"] = build()
    nc = _CACHE["nc"]
    in_maps = shard_inputs(x, labels, W)
    res = run_bass_kernel_spmd(nc, in_maps, core_ids=list(range(CORES)), **run_kwargs)
    out = np.asarray(res.results[0]["out"], dtype=np.float32).reshape(())
    if run_kwargs:
        return out, res
    return out


# revision 23
# speedup vs baseline: 1.1108x; 1.1108x over previous
"""AdMSoftmaxLoss on 8 TRN2 NeuronCores (Bass/Tile).

Math (matches the reference exactly):
    xn    = x / ||x||_row
    wf    = xn @ W.T                      [N, C]
    t_i   = wf[i, y_i]
    num_i = s*(t_i - m)
    den_i = exp(num_i) + sum_j exp(s*wf_ij) - exp(s*t_i)
    loss  = -mean(num_i - log(den_i))

Distribution: vocab/tensor parallel. W's class dim is sharded 8 ways;
each core computes its slice of the logits as an fp8e4 DoubleRow matmul.
x is row-normalized on the host, so the exp scale is a compile-time
constant and PSUM drains have no data dependency on a norms pass.

v2 structure (from trace analysis of v1):
- PSUM per (ni, m) tile is split into two pool tiles, psS (<=3 banks,
  ScalarE table-Exp drain w/ accum_out row-sums) and psD (1 bank, DVE
  Schraudolph exp2 drain). With one shared tile the framework chained
  DVE's psum read behind ScalarE's accumulator drain, making the drain
  chain ~3.2us > the 2.1us GPIO-throttled matmul fill and costing ~1us
  every other tile.
- The target-dot products are single gpsimd scalar_tensor_tensor
  instructions (mult, mult, accum_out=tz). v1 put the dot row-sum
  reduces on DVE, where the first one head-of-line blocked the in-order
  DVE queue for 38us waiting on the SWDGE gathers.
- Class dim padded 12500 -> 12544 (44 pad cols, v1 used 300).
- First W tile is 1024 cols so the matmul ramp starts earlier; xq is
  split across the scalar+vector DMA queues.
- Phase A AllReduce (class tiles 0-4 + target partials) overlaps tiles
  5-6; its epilogue math is precomputed mid-body so the exposed tail is
  only: last drain -> AllReduce B -> den add/log -> output.
"""

import math

import ml_dtypes
import numpy as np

import concourse.bacc as bacc
import concourse.bass_isa as bass_isa
import concourse.bass as bass
import concourse.mybir as mybir
import concourse.tile as tile
from concourse.bass_utils import run_bass_kernel_spmd

N, D, C, CORES = 4096, 512, 100000, 8
CSH = C // CORES
S_SCALE, MARGIN = 30.0, 0.4

F32 = mybir.dt.float32
BF16 = mybir.dt.bfloat16
I32 = mybir.dt.int32
FP8 = mybir.dt.float8e4
AF = mybir.ActivationFunctionType
OP = mybir.AluOpType
AX = mybir.AxisListType
ReduceOp = bass_isa.ReduceOp

XSCALE, WSCALE = 360.0, 256.0
KEXP = S_SCALE / (XSCALE * WSCALE)          # exp scale on raw psum values
A_SCH = (1 << 23) * math.log2(math.e) * KEXP  # Schraudolph multiplier
B_SCH = 1.0648707e9                           # tuned magic constant

# Per-core class dim padded 12500 -> 12544 with zero-weight columns.
# NTL entries: (start, width, ScalarE share width). ScalarE share is a
# multiple of 512 so the psS/psD pool tiles stay PSUM-bank aligned.
CSHP = 12544
# 12544 = 7 uniform tiles of 1792: ScalarE drains 1280 cols/tile (80%
# of the 1.84us GPIO-throttled fill), DVE 512 (74%) -- both engines
# keep slack so psum WAR never stalls the matmul stream.
NTL = [(i * 1792, 1792, 1280) for i in range(7)]
assert NTL[-1][0] + NTL[-1][1] == CSHP
NNT = len(NTL)
MMW = 512  # matmul moving width (ISA limit: 512 psum cols)
# pad cols 12500..12544 sit in the last tile's DVE share (global
# [12032,12544)): each contributes the exactly computable Schraudolph
# constant for input 0.
_C0 = float(np.int32(int(np.float32(0.0) * np.float32(A_SCH)
                         + np.float32(B_SCH))).view(np.float32))
C_PAD_TOTAL = CORES * 44 * _C0

import os
K_NODOT = bool(int(os.environ.get("K_NODOT", "0")))
K_NOGATHER = bool(int(os.environ.get("K_NOGATHER", "0")))


def build(n=N, d=D, csh=CSH, cores=CORES):
    mt, kt2 = n // 128, d // 256
    nc = bacc.Bacc("TRN2", target_bir_lowering=False, debug=False, num_devices=cores)

    # host-packed DoubleRow layouts: [ki, k2, ko, *] with d = k2*256 + ko*128 + ki
    xq_d = nc.dram_tensor("xq", [128, kt2, 2, n], FP8, kind="ExternalInput")
    xn_d = nc.dram_tensor("xn", [128, mt, d], BF16, kind="ExternalInput")
    wq_d = nc.dram_tensor("wq", [128, kt2, 2, CSHP], FP8, kind="ExternalInput")
    # one extra all-zeros row: out-of-shard labels gather it (no memsets)
    wn_d = nc.dram_tensor("wn", [csh + 1, d], BF16, kind="ExternalInput")
    off_d = nc.dram_tensor("off", [128, mt], I32, kind="ExternalInput")
    out_d = nc.dram_tensor("out", [1, 1], F32, kind="ExternalOutput")
    ccA_in = nc.dram_tensor("ccA_in", [128, 2 * mt], F32)
    ccA_out = nc.dram_tensor("ccA_out", [128, 2 * mt], F32, addr_space="Shared")
    ccB_in = nc.dram_tensor("ccB_in", [128, mt], F32)
    ccB_out = nc.dram_tensor("ccB_out", [128, mt], F32, addr_space="Shared")
    barz_d = nc.dram_tensor("barz", [128, 1], F32, kind="ExternalInput")
    bar_in_d = nc.dram_tensor("bar_in", [128, 1], F32)
    bar_out_d = nc.dram_tensor("bar_out", [128, 1], F32, addr_space="Shared")

    with tile.TileContext(nc) as tc:
        with (
            tc.tile_pool(name="const", bufs=1) as cpool,
            tc.tile_pool(name="wstream", bufs=4) as wpool,
            tc.tile_pool(name="escr", bufs=3) as epool,
            tc.tile_pool(name="iscr", bufs=3) as ipool,
            tc.tile_pool(name="dscr", bufs=2) as dpool,
            tc.tile_pool(name="psum", bufs=2, space="PSUM") as ppool,
        ):
            # start-of-kernel barrier: a [128,1] AllReduce of host-zeros
            # whose result becomes the drain ACT bias. Every core's drain
            # pipeline (and via psum WAR, its matmul stream) is gated on
            # global start alignment, under the shadow of the initial
            # DMA loads. Aligned cores -> the phase A/B collectives wait
            # ~2.5us instead of 20-100us on a straggler. The input is an
            # ExternalInput (ready at t=0, no init chain) and the
            # collective is emitted at priority 0 so the tile scheduler
            # keeps it at the head of the gpsimd queue.
            with tc.high_priority():
                # collectives can't read IO tensors: bounce the host
                # zeros through an internal DRAM scratch first
                nc.scalar.dma_start(bar_in_d[:, :], barz_d[:, :])
                nc.gpsimd.collective_compute(
                    "AllReduce", OP.add, replica_groups=[list(range(cores))],
                    ins=[bar_in_d.ap().opt()], outs=[bar_out_d.ap().opt()])

            # stationary x^T (fp8 DoubleRow) on the scalar DMA queue
            xts = cpool.tile([128, kt2, 2, n], FP8, tag="xts")
            nc.scalar.dma_start(xts[:, :, :, :], xq_d[:, :, :, :])
            bias_sb = cpool.tile([128, 1], F32, tag="bias_sb")
            nc.scalar.dma_start(bias_sb[:, :], bar_out_d[:, :])
            xr_all = cpool.tile([128, mt, d], BF16, tag="xr_all")
            # W stream + xn ride the sync queue; xn after the first two W
            # tiles so the matmul ramp isn't starved.
            wts = []
            for ni, (n0, nw, _) in enumerate(NTL):
                wt = wpool.tile([128, kt2, 2, 2048], FP8, tag="wt", name="wt")
                nc.sync.dma_start(wt[:, :, :, :nw], wq_d[:, :, :, n0 : n0 + nw])
                wts.append(wt)
                if ni == 1:
                    nc.sync.dma_start(xr_all[:, :, :], xn_d[:, :, :])

            # target-row gathers + dot products, all on the gpsimd queue.
            # The gather offsets pass through a barrier-gated no-op add
            # (in1 is the all-zeros barrier result): this forces the
            # scheduler to place the barrier collective AHEAD of the
            # ~90us SWDGE gather+fold chain on the in-order gpsimd queue,
            # so it completes at ~15us while still gating the drains.
            off_raw = cpool.tile([128, mt], I32, tag="off_raw")
            nc.gpsimd.dma_start(off_raw[:, :], off_d[:, :])
            off_sb = cpool.tile([128, mt], I32, tag="off")
            nc.gpsimd.tensor_tensor(
                out=off_sb[:, :], in0=off_raw[:, :],
                in1=bias_sb[:, 0:1].bitcast(I32).broadcast_to([128, mt]),
                op=OP.add)
            wy_all = cpool.tile([128, mt, d], BF16, tag="wy_all")
            if not K_NOGATHER:
                for m in range(mt):
                    nc.gpsimd.indirect_dma_start(
                        out=wy_all[:, m, :], out_offset=None, in_=wn_d[:, :],
                        in_offset=bass.IndirectOffsetOnAxis(
                            ap=off_sb[:, m : m + 1], axis=0),
                        bounds_check=csh, oob_is_err=False)

            # gpsimd has no free-axis reduce, so the row dots are a
            # batched binary tree fold (bf16 products -> f32 halves ->
            # f32 folds), never touching the DVE/ScalarE queues.
            tz = cpool.tile([128, mt], F32, tag="tz")
            if K_NODOT:
                nc.vector.memset(tz[:, :], 0.0)
            else:
                pr_all = cpool.tile([128, mt, d], BF16, tag="pr_all")
                for m in range(mt):
                    nc.gpsimd.tensor_tensor(
                        out=pr_all[:, m, :], in0=xr_all[:, m, :],
                        in1=wy_all[:, m, :], op=OP.mult)
                prh = cpool.tile([128, mt, d // 2], F32, tag="prh")
                nc.gpsimd.tensor_tensor(
                    out=prh[:, :, :], in0=pr_all[:, :, 0 : d // 2],
                    in1=pr_all[:, :, d // 2 : d], op=OP.add)
                w = d // 4
                while w >= 1:
                    nc.gpsimd.tensor_tensor(
                        out=prh[:, :, 0:w], in0=prh[:, :, 0:w],
                        in1=prh[:, :, w : 2 * w], op=OP.add)
                    w //= 2
                nc.gpsimd.tensor_copy(out=tz[:, :], in_=prh[:, :, 0])

            # per-engine accumulators (separate tiles so ScalarE accum
            # writes and DVE reduce writes never WAW-serialize)
            accS = cpool.tile([128, mt, NNT], F32, tag="accS")
            accD = cpool.tile([128, mt, NNT], F32, tag="accD")

            lnc = cpool.tile([128, 1], F32, tag="lnc")
            nc.vector.memset(
                lnc[:, :], float(math.log(1.0 - math.exp(-S_SCALE * MARGIN))))
            csm = cpool.tile([128, 1], F32, tag="csm")
            nc.vector.memset(csm[:, :], float(S_SCALE * MARGIN))

            # main loop: fp8 DoubleRow matmuls into per-engine psum tiles
            # (psS: ScalarE table Exp w/ free accum row-sum; psD: DVE
            # Schraudolph exp2 via i32 affine + bitcast + reduce).
            NA = 5  # ntiles in collective phase A (classes [0, 9216))
            rr = cpool.tile([128, 2 * mt], F32, tag="rr")
            Bs = cpool.tile([128, 2], F32, tag="Bs")
            denA = cpool.tile([128, mt], F32, tag="denA")
            for ni, (n0, nw, scw) in enumerate(NTL):
                wt = wts[ni]
                dvw = nw - scw
                # chunk so no matmul write crosses a PSUM bank boundary
                # (psS/psD tiles are bank-aligned; scw itself need not
                # be a multiple of 512)
                nsub = [(j0, min(MMW, scw - j0)) for j0 in range(0, scw, MMW)]
                nsub += [(scw + j0, min(MMW, dvw - j0))
                         for j0 in range(0, dvw, MMW)]
                for m in range(mt):
                    psS = ppool.tile([128, 1536], F32, tag="psS", name="psS")
                    psD = ppool.tile([128, 512], F32, tag="psD", name="psD")
                    for k2 in range(kt2):
                        for j0, jw in nsub:
                            if j0 < scw:
                                dst = psS[:, j0 : j0 + jw]
                            else:
                                dst = psD[:, j0 - scw : j0 - scw + jw]
                            nc.tensor.matmul(
                                out=dst,
                                lhsT=xts[:, k2, :, m * 128 : (m + 1) * 128],
                                rhs=wt[:, k2, :, j0 : j0 + jw],
                                start=(k2 == 0), stop=(k2 == kt2 - 1),
                                perf_mode=mybir.MatmulPerfMode.DoubleRow)
                    # ScalarE share: exp -> bf16 with free row-sum accum
                    # (bias is the all-zeros barrier result: math no-op,
                    # gates the drain stream on the start barrier)
                    et = epool.tile([128, 1536], BF16, tag="et", name="et")
                    nc.scalar.activation(
                        et[:, :scw], psS[:, :scw], AF.Exp,
                        bias=bias_sb[:, 0:1], scale=KEXP,
                        accum_out=accS[:, m, ni : ni + 1])
                    # DVE share: Schraudolph exp2 (i32 convert + bitcast)
                    ti = ipool.tile([128, 512], I32, tag="ti", name="ti")
                    nc.vector.tensor_scalar(
                        out=ti[:, :dvw], in0=psD[:, :dvw],
                        scalar1=float(A_SCH), scalar2=float(B_SCH),
                        op0=OP.mult, op1=OP.add)
                    nc.vector.tensor_reduce(
                        out=accD[:, m, ni : ni + 1],
                        in_=ti[:, :dvw].bitcast(F32), axis=AX.X, op=OP.add)
                if ni == NA - 1:
                    # phase A: expsum partials for classes [0, 9216) + all
                    # target-logit partials; AllReduce overlaps ni 5-6.
                    ccsbA = cpool.tile([128, 2 * mt], F32, tag="ccsbA")
                    tmpA = cpool.tile([128, mt], F32, tag="tmpA")
                    nc.vector.tensor_reduce(
                        out=ccsbA[:, 0:mt], in_=accS[:, :, 0:NA],
                        axis=AX.X, op=OP.add)
                    nc.vector.tensor_reduce(
                        out=tmpA[:, :], in_=accD[:, :, 0:NA], axis=AX.X, op=OP.add)
                    nc.vector.tensor_tensor(
                        out=ccsbA[:, 0:mt], in0=ccsbA[:, 0:mt], in1=tmpA[:, :],
                        op=OP.add)
                    nc.vector.tensor_copy(out=ccsbA[:, mt : 2 * mt], in_=tz[:, :])
                    nc.sync.dma_start(ccA_in[:, :], ccsbA[:, :])
                    nc.gpsimd.collective_compute(
                        "AllReduce", OP.add,
                        replica_groups=[list(range(cores))],
                        ins=[ccA_in.ap().opt()], outs=[ccA_out.ap().opt()])
                if ni == NA:
                    # phase-A result readback (sync queue; waits on the
                    # collective, but nothing downstream consumes it
                    # until the tail, so no engine queue blocks on it)
                    nc.sync.dma_start(rr[:, :], ccA_out[:, :])

            # phase B: expsum partials for classes [9216, 12544)
            ccsbB = cpool.tile([128, mt], F32, tag="ccsbB")
            tmpB = cpool.tile([128, mt], F32, tag="tmpB")
            nc.vector.tensor_reduce(
                out=ccsbB[:, 0:mt], in_=accS[:, :, NA:NNT], axis=AX.X, op=OP.add)
            nc.vector.tensor_reduce(
                out=tmpB[:, :], in_=accD[:, :, NA:NNT], axis=AX.X, op=OP.add)
            nc.vector.tensor_tensor(
                out=ccsbB[:, 0:mt], in0=ccsbB[:, 0:mt], in1=tmpB[:, :], op=OP.add)
            nc.sync.dma_start(ccB_in[:, :], ccsbB[:, :])
            nc.gpsimd.collective_compute(
                "AllReduce", OP.add, replica_groups=[list(range(cores))],
                ins=[ccB_in.ap().opt()], outs=[ccB_out.ap().opt()])

            # phase-A epilogue (emitted at the tail so it can't
            # head-of-line block the Scalar/Vector queues behind the
            # collective): B = s*t_tot; Bs0 = sum(B);
            # Es = exp(B + ln(1-e^{-sm})); denA = expsumA - Es - pads
            B = cpool.tile([128, mt], F32, tag="B")
            nc.vector.tensor_scalar(
                out=B[:, :], in0=rr[:, mt : 2 * mt], scalar1=S_SCALE,
                scalar2=None, op0=OP.mult, op1=OP.add, accum_out=Bs[:, 0:1])
            Es = cpool.tile([128, mt], F32, tag="Es")
            nc.scalar.activation(
                Es[:, :], B[:, :], AF.Exp, bias=lnc[:, :1], scale=1.0)
            nc.vector.tensor_tensor(
                out=denA[:, :], in0=rr[:, 0:mt], in1=Es[:, :], op=OP.subtract)
            nc.vector.tensor_scalar(
                out=denA[:, :], in0=denA[:, :], scalar1=float(C_PAD_TOTAL),
                scalar2=None, op0=OP.subtract)

            rrB = cpool.tile([128, mt], F32, tag="rrB")
            nc.sync.dma_start(rrB[:, :], ccB_out[:, :])

            # exposed tail: den = denA + expsumB; loss = s*m - (sum(B) -
            # sum(ln den))/n
            den = cpool.tile([128, mt], F32, tag="den")
            nc.vector.tensor_tensor(
                out=den[:, :], in0=denA[:, :], in1=rrB[:, :], op=OP.add)
            lden = cpool.tile([128, mt], F32, tag="lden")
            nc.scalar.activation(
                lden[:, :], den[:, :], AF.Ln, accum_out=Bs[:, 1:2])
            diff = cpool.tile([128, 1], F32, tag="diff")
            nc.vector.tensor_tensor(
                out=diff[:, :], in0=Bs[:, 0:1], in1=Bs[:, 1:2], op=OP.subtract)
            zs = cpool.tile([128, 1], F32, tag="zs")
            nc.gpsimd.partition_all_reduce(zs[:, :], diff[:, :], 128, ReduceOp.add)
            res = cpool.tile([1, 1], F32, tag="res")
            nc.scalar.activation(
                res[:, :], zs[:1, :], AF.Identity,
                bias=csm[:1, :], scale=-1.0 / n)
            nc.sync.dma_start(out_d[:, :], res[:, :])
    nc.compile()
    return nc


def shard_inputs(x, labels, W, n=N, d=D, csh=CSH, cores=CORES):
    x32 = np.ascontiguousarray(np.asarray(x), dtype=np.float32)
    xn = x32 / np.sqrt((x32 * x32).sum(axis=1, keepdims=True))
    # [ki, k2, ko, n] fp8 DoubleRow layout: d = k2*256 + ko*128 + ki
    xq = (xn.T * XSCALE).reshape(2, 2, 128, n).transpose(2, 0, 1, 3)
    xq = np.ascontiguousarray(np.clip(xq, -240, 240)).astype(ml_dtypes.float8_e4m3)
    # [p, m, d] bf16 rows
    xnb = np.ascontiguousarray(
        xn.reshape(n // 128, 128, d).transpose(1, 0, 2)).astype(ml_dtypes.bfloat16)
    lab = np.asarray(labels).astype(np.int64).reshape(n)
    in_maps = []
    for r in range(cores):
        Wc = np.ascontiguousarray(np.asarray(W)[r * csh : (r + 1) * csh], dtype=np.float32)
        Wp = np.zeros((CSHP, d), np.float32)
        Wp[:csh] = Wc
        wq = (Wp.T * WSCALE).reshape(2, 2, 128, CSHP).transpose(2, 0, 1, 3)
        wq = np.ascontiguousarray(np.clip(wq, -240, 240)).astype(ml_dtypes.float8_e4m3)
        loc = lab - r * csh
        off = np.where((loc >= 0) & (loc < csh), loc, csh).astype(np.int32)
        off = np.ascontiguousarray(off.reshape(n // 128, 128).T)
        wn = np.vstack([Wc, np.zeros((1, d), np.float32)]).astype(ml_dtypes.bfloat16)
        in_maps.append({
            "xq": xq, "xn": xnb, "wq": wq,
            "wn": np.ascontiguousarray(wn), "off": off,
            "barz": np.zeros((128, 1), np.float32),
        })
    return in_maps


_CACHE = {}


def kernel(x, labels, W, **run_kwargs):
    if "nc" not in _CACHE:
        _CACHE["nc"] = build()
    nc = _CACHE["nc"]
    in_maps = shard_inputs(x, labels, W)
    res = run_bass_kernel_spmd(nc, in_maps, core_ids=list(range(CORES)), **run_kwargs)
    out = np.asarray(res.results[0]["out"], dtype=np.float32).reshape(())
    if run_kwargs:
        return out, res
    return out


# revision 28
# speedup vs baseline: 1.2241x; 1.1020x over previous
"""AdMSoftmaxLoss on 8 TRN2 NeuronCores (Bass/Tile).

Math (matches the reference exactly):
    xn    = x / ||x||_row
    wf    = xn @ W.T                      [N, C]
    t_i   = wf[i, y_i]
    num_i = s*(t_i - m)
    den_i = exp(num_i) + sum_j exp(s*wf_ij) - exp(s*t_i)
    loss  = -mean(num_i - log(den_i))

Distribution: vocab/tensor parallel. W's class dim is sharded 8 ways;
each core computes its slice of the logits as an fp8e4 DoubleRow matmul.
x is row-normalized on the host, so the exp scale is a compile-time
constant and PSUM drains have no data dependency on a norms pass.

v2 structure (from trace analysis of v1):
- PSUM per (ni, m) tile is split into two pool tiles, psS (<=3 banks,
  ScalarE table-Exp drain w/ accum_out row-sums) and psD (1 bank, DVE
  Schraudolph exp2 drain). With one shared tile the framework chained
  DVE's psum read behind ScalarE's accumulator drain, making the drain
  chain ~3.2us > the 2.1us GPIO-throttled matmul fill and costing ~1us
  every other tile.
- The target-dot products are single gpsimd scalar_tensor_tensor
  instructions (mult, mult, accum_out=tz). v1 put the dot row-sum
  reduces on DVE, where the first one head-of-line blocked the in-order
  DVE queue for 38us waiting on the SWDGE gathers.
- Class dim padded 12500 -> 12544 (44 pad cols, v1 used 300).
- First W tile is 1024 cols so the matmul ramp starts earlier; xq is
  split across the scalar+vector DMA queues.
- Phase A AllReduce (class tiles 0-4 + target partials) overlaps tiles
  5-6; its epilogue math is precomputed mid-body so the exposed tail is
  only: last drain -> AllReduce B -> den add/log -> output.
"""

import math

import ml_dtypes
import numpy as np

import concourse.bacc as bacc
import concourse.bass_isa as bass_isa
import concourse.bass as bass
import concourse.mybir as mybir
import concourse.tile as tile
from concourse.bass_utils import run_bass_kernel_spmd

N, D, C, CORES = 4096, 512, 100000, 8
CSH = C // CORES
S_SCALE, MARGIN = 30.0, 0.4

F32 = mybir.dt.float32
BF16 = mybir.dt.bfloat16
I32 = mybir.dt.int32
FP8 = mybir.dt.float8e4
AF = mybir.ActivationFunctionType
OP = mybir.AluOpType
AX = mybir.AxisListType
ReduceOp = bass_isa.ReduceOp

XSCALE, WSCALE = 360.0, 256.0
KEXP = S_SCALE / (XSCALE * WSCALE)          # exp scale on raw psum values
A_SCH = (1 << 23) * math.log2(math.e) * KEXP  # Schraudolph multiplier
B_SCH = 1.0648707e9                           # tuned magic constant

# Per-core class dim padded 12500 -> 12544 with zero-weight columns.
# NTL entries: (start, width, ScalarE share width). ScalarE share is a
# multiple of 512 so the psS/psD pool tiles stay PSUM-bank aligned.
CSHP = 12544
# 12544 = 7 uniform tiles of 1792: ScalarE drains 1280 cols/tile (80%
# of the 1.84us GPIO-throttled fill), DVE 512 (74%) -- both engines
# keep slack so psum WAR never stalls the matmul stream.
NTL = [(i * 1792, 1792, 1280) for i in range(7)]
assert NTL[-1][0] + NTL[-1][1] == CSHP
NNT = len(NTL)
MMW = 512  # matmul moving width (ISA limit: 512 psum cols)
# pad cols 12500..12544 sit in the last tile's DVE share (global
# [12032,12544)): each contributes the exactly computable Schraudolph
# constant for input 0.
_C0 = float(np.int32(int(np.float32(0.0) * np.float32(A_SCH)
                         + np.float32(B_SCH))).view(np.float32))
C_PAD_TOTAL = CORES * 44 * _C0

import os
K_NODOT = bool(int(os.environ.get("K_NODOT", "0")))
K_NOGATHER = bool(int(os.environ.get("K_NOGATHER", "0")))


def build(n=N, d=D, csh=CSH, cores=CORES):
    mt, kt2 = n // 128, d // 256
    nc = bacc.Bacc("TRN2", target_bir_lowering=False, debug=False, num_devices=cores)

    # host-packed DoubleRow layouts: [ki, k2, ko, *] with d = k2*256 + ko*128 + ki
    xq_d = nc.dram_tensor("xq", [128, kt2, 2, n], FP8, kind="ExternalInput")
    xn_d = nc.dram_tensor("xn", [128, mt, d], BF16, kind="ExternalInput")
    wq_d = nc.dram_tensor("wq", [128, kt2, 2, CSHP], FP8, kind="ExternalInput")
    # one extra all-zeros row: out-of-shard labels gather it (no memsets)
    wn_d = nc.dram_tensor("wn", [csh + 1, d], BF16, kind="ExternalInput")
    off_d = nc.dram_tensor("off", [128, mt], I32, kind="ExternalInput")
    out_d = nc.dram_tensor("out", [1, 1], F32, kind="ExternalOutput")
    ccA_in = nc.dram_tensor("ccA_in", [128, 2 * mt], F32)
    ccA_out = nc.dram_tensor("ccA_out", [128, 2 * mt], F32, addr_space="Shared")
    ccB_in = nc.dram_tensor("ccB_in", [128, mt], F32)
    ccB_out = nc.dram_tensor("ccB_out", [128, mt], F32, addr_space="Shared")


    with tile.TileContext(nc) as tc:
        with (
            tc.tile_pool(name="const", bufs=1) as cpool,
            tc.tile_pool(name="wstream", bufs=4) as wpool,
            tc.tile_pool(name="escr", bufs=3) as epool,
            tc.tile_pool(name="iscr", bufs=3) as ipool,
            tc.tile_pool(name="dscr", bufs=2) as dpool,
            tc.tile_pool(name="psum", bufs=2, space="PSUM") as ppool,
        ):
            # stationary x^T (fp8 DoubleRow) on the scalar DMA queue
            xts = cpool.tile([128, kt2, 2, n], FP8, tag="xts")
            nc.scalar.dma_start(xts[:, :, :, :], xq_d[:, :, :, :])
            xr_all = cpool.tile([128, mt, d], BF16, tag="xr_all")
            # W stream + xn ride the sync queue; xn after the first two W
            # tiles so the matmul ramp isn't starved.
            wts = []
            for ni, (n0, nw, _) in enumerate(NTL):
                wt = wpool.tile([128, kt2, 2, 2048], FP8, tag="wt", name="wt")
                nc.sync.dma_start(wt[:, :, :, :nw], wq_d[:, :, :, n0 : n0 + nw])
                wts.append(wt)
                if ni == 1:
                    nc.sync.dma_start(xr_all[:, :, :], xn_d[:, :, :])

            # target-row gathers + dot products, all on the gpsimd queue:
            # 32 indirect SWDGE gathers, then the batched dot tree.
            # Nothing here touches the DVE/ScalarE queues.
            off_sb = cpool.tile([128, mt], I32, tag="off")
            nc.gpsimd.dma_start(off_sb[:, :], off_d[:, :])
            wy_all = cpool.tile([128, mt, d], BF16, tag="wy_all")
            if not K_NOGATHER:
                for m in range(mt):
                    nc.gpsimd.indirect_dma_start(
                        out=wy_all[:, m, :], out_offset=None, in_=wn_d[:, :],
                        in_offset=bass.IndirectOffsetOnAxis(
                            ap=off_sb[:, m : m + 1], axis=0),
                        bounds_check=csh, oob_is_err=False)

            # gpsimd has no free-axis reduce, so the row dots are a
            # batched binary tree fold (bf16 products -> f32 halves ->
            # f32 folds), never touching the DVE/ScalarE queues.
            tz = cpool.tile([128, mt], F32, tag="tz")
            if K_NODOT:
                nc.vector.memset(tz[:, :], 0.0)
            else:
                pr_all = cpool.tile([128, mt, d], BF16, tag="pr_all")
                for m in range(mt):
                    nc.gpsimd.tensor_tensor(
                        out=pr_all[:, m, :], in0=xr_all[:, m, :],
                        in1=wy_all[:, m, :], op=OP.mult)
                prh = cpool.tile([128, mt, d // 2], F32, tag="prh")
                nc.gpsimd.tensor_tensor(
                    out=prh[:, :, :], in0=pr_all[:, :, 0 : d // 2],
                    in1=pr_all[:, :, d // 2 : d], op=OP.add)
                w = d // 4
                while w >= 1:
                    nc.gpsimd.tensor_tensor(
                        out=prh[:, :, 0:w], in0=prh[:, :, 0:w],
                        in1=prh[:, :, w : 2 * w], op=OP.add)
                    w //= 2
                nc.gpsimd.tensor_copy(out=tz[:, :], in_=prh[:, :, 0])

            # per-engine accumulators (separate tiles so ScalarE accum
            # writes and DVE reduce writes never WAW-serialize)
            accS = cpool.tile([128, mt, NNT], F32, tag="accS")
            accD = cpool.tile([128, mt, NNT], F32, tag="accD")

            lnc = cpool.tile([128, 1], F32, tag="lnc")
            nc.vector.memset(
                lnc[:, :], float(math.log(1.0 - math.exp(-S_SCALE * MARGIN))))
            csm = cpool.tile([128, 1], F32, tag="csm")
            nc.vector.memset(csm[:, :], float(S_SCALE * MARGIN))

            # main loop: fp8 DoubleRow matmuls into per-engine psum tiles
            # (psS: ScalarE table Exp w/ free accum row-sum; psD: DVE
            # Schraudolph exp2 via i32 affine + bitcast + reduce).
            NA = 5  # ntiles in collective phase A (classes [0, 9216))
            rr = cpool.tile([128, 2 * mt], F32, tag="rr")
            Bs = cpool.tile([128, 2], F32, tag="Bs")
            denA = cpool.tile([128, mt], F32, tag="denA")
            for ni, (n0, nw, scw) in enumerate(NTL):
                wt = wts[ni]
                dvw = nw - scw
                # chunk so no matmul write crosses a PSUM bank boundary
                # (psS/psD tiles are bank-aligned; scw itself need not
                # be a multiple of 512)
                nsub = [(j0, min(MMW, scw - j0)) for j0 in range(0, scw, MMW)]
                nsub += [(scw + j0, min(MMW, dvw - j0))
                         for j0 in range(0, dvw, MMW)]
                for m in range(mt):
                    psS = ppool.tile([128, 1536], F32, tag="psS", name="psS")
                    psD = ppool.tile([128, 512], F32, tag="psD", name="psD")
                    for k2 in range(kt2):
                        for j0, jw in nsub:
                            if j0 < scw:
                                dst = psS[:, j0 : j0 + jw]
                            else:
                                dst = psD[:, j0 - scw : j0 - scw + jw]
                            nc.tensor.matmul(
                                out=dst,
                                lhsT=xts[:, k2, :, m * 128 : (m + 1) * 128],
                                rhs=wt[:, k2, :, j0 : j0 + jw],
                                start=(k2 == 0), stop=(k2 == kt2 - 1),
                                perf_mode=mybir.MatmulPerfMode.DoubleRow)
                    # ScalarE share: exp -> bf16 with free row-sum accum
                    et = epool.tile([128, 1536], BF16, tag="et", name="et")
                    nc.scalar.activation(
                        et[:, :scw], psS[:, :scw], AF.Exp, bias=0.0, scale=KEXP,
                        accum_out=accS[:, m, ni : ni + 1])
                    # DVE share: Schraudolph exp2 (i32 convert + bitcast)
                    ti = ipool.tile([128, 512], I32, tag="ti", name="ti")
                    nc.vector.tensor_scalar(
                        out=ti[:, :dvw], in0=psD[:, :dvw],
                        scalar1=float(A_SCH), scalar2=float(B_SCH),
                        op0=OP.mult, op1=OP.add)
                    nc.vector.tensor_reduce(
                        out=accD[:, m, ni : ni + 1],
                        in_=ti[:, :dvw].bitcast(F32), axis=AX.X, op=OP.add)
                if ni == NA - 1:
                    # phase A: expsum partials for classes [0, 9216) + all
                    # target-logit partials; AllReduce overlaps ni 5-6.
                    ccsbA = cpool.tile([128, 2 * mt], F32, tag="ccsbA")
                    tmpA = cpool.tile([128, mt], F32, tag="tmpA")
                    nc.vector.tensor_reduce(
                        out=ccsbA[:, 0:mt], in_=accS[:, :, 0:NA],
                        axis=AX.X, op=OP.add)
                    nc.vector.tensor_reduce(
                        out=tmpA[:, :], in_=accD[:, :, 0:NA], axis=AX.X, op=OP.add)
                    nc.vector.tensor_tensor(
                        out=ccsbA[:, 0:mt], in0=ccsbA[:, 0:mt], in1=tmpA[:, :],
                        op=OP.add)
                    nc.vector.tensor_copy(out=ccsbA[:, mt : 2 * mt], in_=tz[:, :])
                    nc.sync.dma_start(ccA_in[:, :], ccsbA[:, :])
                    nc.gpsimd.collective_compute(
                        "AllReduce", OP.add,
                        replica_groups=[list(range(cores))],
                        ins=[ccA_in.ap().opt()], outs=[ccA_out.ap().opt()])
                if ni == NA:
                    # phase-A result readback (sync queue; waits on the
                    # collective, but nothing downstream consumes it
                    # until the tail, so no engine queue blocks on it)
                    nc.sync.dma_start(rr[:, :], ccA_out[:, :])

            # phase B: expsum partials for classes [9216, 12544)
            ccsbB = cpool.tile([128, mt], F32, tag="ccsbB")
            tmpB = cpool.tile([128, mt], F32, tag="tmpB")
            nc.vector.tensor_reduce(
                out=ccsbB[:, 0:mt], in_=accS[:, :, NA:NNT], axis=AX.X, op=OP.add)
            nc.vector.tensor_reduce(
                out=tmpB[:, :], in_=accD[:, :, NA:NNT], axis=AX.X, op=OP.add)
            nc.vector.tensor_tensor(
                out=ccsbB[:, 0:mt], in0=ccsbB[:, 0:mt], in1=tmpB[:, :], op=OP.add)
            nc.sync.dma_start(ccB_in[:, :], ccsbB[:, :])
            nc.gpsimd.collective_compute(
                "AllReduce", OP.add, replica_groups=[list(range(cores))],
                ins=[ccB_in.ap().opt()], outs=[ccB_out.ap().opt()])

            # phase-A epilogue (emitted at the tail so it can't
            # head-of-line block the Scalar/Vector queues behind the
            # collective): B = s*t_tot; Bs0 = sum(B);
            # Es = exp(B + ln(1-e^{-sm})); denA = expsumA - Es - pads
            B = cpool.tile([128, mt], F32, tag="B")
            nc.vector.tensor_scalar(
                out=B[:, :], in0=rr[:, mt : 2 * mt], scalar1=S_SCALE,
                scalar2=None, op0=OP.mult, op1=OP.add, accum_out=Bs[:, 0:1])
            Es = cpool.tile([128, mt], F32, tag="Es")
            nc.scalar.activation(
                Es[:, :], B[:, :], AF.Exp, bias=lnc[:, :1], scale=1.0)
            nc.vector.tensor_tensor(
                out=denA[:, :], in0=rr[:, 0:mt], in1=Es[:, :], op=OP.subtract)
            nc.vector.tensor_scalar(
                out=denA[:, :], in0=denA[:, :], scalar1=float(C_PAD_TOTAL),
                scalar2=None, op0=OP.subtract)

            rrB = cpool.tile([128, mt], F32, tag="rrB")
            nc.sync.dma_start(rrB[:, :], ccB_out[:, :])

            # exposed tail: den = denA + expsumB; loss = s*m - (sum(B) -
            # sum(ln den))/n
            den = cpool.tile([128, mt], F32, tag="den")
            nc.vector.tensor_tensor(
                out=den[:, :], in0=denA[:, :], in1=rrB[:, :], op=OP.add)
            lden = cpool.tile([128, mt], F32, tag="lden")
            nc.scalar.activation(
                lden[:, :], den[:, :], AF.Ln, accum_out=Bs[:, 1:2])
            diff = cpool.tile([128, 1], F32, tag="diff")
            nc.vector.tensor_tensor(
                out=diff[:, :], in0=Bs[:, 0:1], in1=Bs[:, 1:2], op=OP.subtract)
            zs = cpool.tile([128, 1], F32, tag="zs")
            nc.gpsimd.partition_all_reduce(zs[:, :], diff[:, :], 128, ReduceOp.add)
            res = cpool.tile([1, 1], F32, tag="res")
            nc.scalar.activation(
                res[:, :], zs[:1, :], AF.Identity,
                bias=csm[:1, :], scale=-1.0 / n)
            nc.sync.dma_start(out_d[:, :], res[:, :])
    nc.compile()
    return nc


def shard_inputs(x, labels, W, n=N, d=D, csh=CSH, cores=CORES):
    x32 = np.ascontiguousarray(np.asarray(x), dtype=np.float32)
    xn = x32 / np.sqrt((x32 * x32).sum(axis=1, keepdims=True))
    # [ki, k2, ko, n] fp8 DoubleRow layout: d = k2*256 + ko*128 + ki
    xq = (xn.T * XSCALE).reshape(2, 2, 128, n).transpose(2, 0, 1, 3)
    xq = np.ascontiguousarray(np.clip(xq, -240, 240)).astype(ml_dtypes.float8_e4m3)
    # [p, m, d] bf16 rows
    xnb = np.ascontiguousarray(
        xn.reshape(n // 128, 128, d).transpose(1, 0, 2)).astype(ml_dtypes.bfloat16)
    lab = np.asarray(labels).astype(np.int64).reshape(n)
    in_maps = []
    for r in range(cores):
        Wc = np.ascontiguousarray(np.asarray(W)[r * csh : (r + 1) * csh], dtype=np.float32)
        Wp = np.zeros((CSHP, d), np.float32)
        Wp[:csh] = Wc
        wq = (Wp.T * WSCALE).reshape(2, 2, 128, CSHP).transpose(2, 0, 1, 3)
        wq = np.ascontiguousarray(np.clip(wq, -240, 240)).astype(ml_dtypes.float8_e4m3)
        loc = lab - r * csh
        off = np.where((loc >= 0) & (loc < csh), loc, csh).astype(np.int32)
        off = np.ascontiguousarray(off.reshape(n // 128, 128).T)
        wn = np.vstack([Wc, np.zeros((1, d), np.float32)]).astype(ml_dtypes.bfloat16)
        in_maps.append({
            "xq": xq, "xn": xnb, "wq": wq,
            "wn": np.ascontiguousarray(wn), "off": off,
        })
    return in_maps


_CACHE = {}


def kernel(x, labels, W, **run_kwargs):
    if "nc" not in _CACHE:
        _CACHE["nc"] = build()
    nc = _CACHE["nc"]
    in_maps = shard_inputs(x, labels, W)
    res = run_bass_kernel_spmd(nc, in_maps, core_ids=list(range(CORES)), **run_kwargs)
    out = np.asarray(res.results[0]["out"], dtype=np.float32).reshape(())
    if run_kwargs:
        return out, res
    return out


# revision 32
# speedup vs baseline: 1.2263x; 1.0018x over previous
"""AdMSoftmaxLoss on 8 TRN2 NeuronCores (Bass/Tile).

Math (matches the reference exactly):
    xn    = x / ||x||_row
    wf    = xn @ W.T                      [N, C]
    t_i   = wf[i, y_i]
    num_i = s*(t_i - m)
    den_i = exp(num_i) + sum_j exp(s*wf_ij) - exp(s*t_i)
    loss  = -mean(num_i - log(den_i))

Distribution: vocab/tensor parallel. W's class dim is sharded 8 ways;
each core computes its slice of the logits as an fp8e4 DoubleRow matmul.
x is row-normalized on the host, so the exp scale is a compile-time
constant and PSUM drains have no data dependency on a norms pass.

v2 structure (from trace analysis of v1):
- PSUM per (ni, m) tile is split into two pool tiles, psS (<=3 banks,
  ScalarE table-Exp drain w/ accum_out row-sums) and psD (1 bank, DVE
  Schraudolph exp2 drain). With one shared tile the framework chained
  DVE's psum read behind ScalarE's accumulator drain, making the drain
  chain ~3.2us > the 2.1us GPIO-throttled matmul fill and costing ~1us
  every other tile.
- The target-dot products are single gpsimd scalar_tensor_tensor
  instructions (mult, mult, accum_out=tz). v1 put the dot row-sum
  reduces on DVE, where the first one head-of-line blocked the in-order
  DVE queue for 38us waiting on the SWDGE gathers.
- Class dim padded 12500 -> 12544 (44 pad cols, v1 used 300).
- First W tile is 1024 cols so the matmul ramp starts earlier; xq is
  split across the scalar+vector DMA queues.
- Phase A AllReduce (class tiles 0-4 + target partials) overlaps tiles
  5-6; its epilogue math is precomputed mid-body so the exposed tail is
  only: last drain -> AllReduce B -> den add/log -> output.
"""

import math

import ml_dtypes
import numpy as np

import concourse.bacc as bacc
import concourse.bass_isa as bass_isa
import concourse.bass as bass
import concourse.mybir as mybir
import concourse.tile as tile
from concourse.bass_utils import run_bass_kernel_spmd

N, D, C, CORES = 4096, 512, 100000, 8
CSH = C // CORES
S_SCALE, MARGIN = 30.0, 0.4

F32 = mybir.dt.float32
BF16 = mybir.dt.bfloat16
I32 = mybir.dt.int32
I16 = mybir.dt.int16
FP8 = mybir.dt.float8e4
AF = mybir.ActivationFunctionType
OP = mybir.AluOpType
AX = mybir.AxisListType
ReduceOp = bass_isa.ReduceOp

XSCALE, WSCALE = 360.0, 256.0
KEXP = S_SCALE / (XSCALE * WSCALE)          # exp scale on raw psum values
# bf16 Schraudolph: i16 codes (top 16 bits of the f32 trick) -> bitcast
# bf16. Halves the DVE reduce cost (2x-mode bf16 TENSOR_REDUCE); the
# coarser exp approximation is zero-mean and covers only ~28% of the
# expsum, noise-level for the final loss.
A_SCH = (1 << 7) * math.log2(math.e) * KEXP   # Schraudolph multiplier
B_SCH = 1.0648707e9 / 65536.0                 # tuned magic constant

# Per-core class dim padded 12500 -> 12544 with zero-weight columns.
# NTL entries: (start, width, ScalarE share width). ScalarE share is a
# multiple of 512 so the psS/psD pool tiles stay PSUM-bank aligned.
CSHP = 12544
# 12544 = 7 uniform tiles of 1792: ScalarE drains 1280 cols/tile (80%
# of the 1.84us GPIO-throttled fill), DVE 512 (74%) -- both engines
# keep slack so psum WAR never stalls the matmul stream.
NTL = [(i * 1792, 1792, 1280) for i in range(7)]
assert NTL[-1][0] + NTL[-1][1] == CSHP
NNT = len(NTL)
MMW = 512  # matmul moving width (ISA limit: 512 psum cols)
# pad cols 12500..12544 sit in the last tile's DVE share (global
# [12032,12544)): each contributes the exactly computable Schraudolph
# constant for input 0.
_C0 = float(np.array(int(np.float32(0.0) * np.float32(A_SCH)
                         + np.float32(B_SCH)),
                     np.int16).view(ml_dtypes.bfloat16))
C_PAD_TOTAL = CORES * 44 * _C0

import os
K_NODOT = bool(int(os.environ.get("K_NODOT", "0")))
K_NOGATHER = bool(int(os.environ.get("K_NOGATHER", "0")))


def build(n=N, d=D, csh=CSH, cores=CORES):
    mt, kt2 = n // 128, d // 256
    nc = bacc.Bacc("TRN2", target_bir_lowering=False, debug=False, num_devices=cores)

    # host-packed DoubleRow layouts: [ki, k2, ko, *] with d = k2*256 + ko*128 + ki
    xq_d = nc.dram_tensor("xq", [128, kt2, 2, n], FP8, kind="ExternalInput")
    xn_d = nc.dram_tensor("xn", [128, mt, d], BF16, kind="ExternalInput")
    wq_d = nc.dram_tensor("wq", [128, kt2, 2, CSHP], FP8, kind="ExternalInput")
    # one extra all-zeros row: out-of-shard labels gather it (no memsets)
    wn_d = nc.dram_tensor("wn", [csh + 1, d], BF16, kind="ExternalInput")
    off_d = nc.dram_tensor("off", [128, mt], I32, kind="ExternalInput")
    out_d = nc.dram_tensor("out", [1, 1], F32, kind="ExternalOutput")
    ccA_in = nc.dram_tensor("ccA_in", [128, 2 * mt], F32)
    ccA_out = nc.dram_tensor("ccA_out", [128, 2 * mt], F32, addr_space="Shared")
    ccB_in = nc.dram_tensor("ccB_in", [128, mt], F32)
    ccB_out = nc.dram_tensor("ccB_out", [128, mt], F32, addr_space="Shared")


    with tile.TileContext(nc) as tc:
        with (
            tc.tile_pool(name="const", bufs=1) as cpool,
            tc.tile_pool(name="wstream", bufs=4) as wpool,
            tc.tile_pool(name="escr", bufs=3) as epool,
            tc.tile_pool(name="iscr", bufs=3) as ipool,
            tc.tile_pool(name="dscr", bufs=2) as dpool,
            tc.tile_pool(name="psum", bufs=2, space="PSUM") as ppool,
        ):
            # stationary x^T (fp8 DoubleRow) on the scalar DMA queue
            xts = cpool.tile([128, kt2, 2, n], FP8, tag="xts")
            nc.scalar.dma_start(xts[:, :, :, :], xq_d[:, :, :, :])
            xr_all = cpool.tile([128, mt, d], BF16, tag="xr_all")
            # W stream + xn ride the sync queue; xn after the first two W
            # tiles so the matmul ramp isn't starved.
            wts = []
            for ni, (n0, nw, _) in enumerate(NTL):
                wt = wpool.tile([128, kt2, 2, 2048], FP8, tag="wt", name="wt")
                nc.sync.dma_start(wt[:, :, :, :nw], wq_d[:, :, :, n0 : n0 + nw])
                wts.append(wt)
                if ni == 1:
                    nc.sync.dma_start(xr_all[:, :, :], xn_d[:, :, :])

            # target-row gathers + dot products, all on the gpsimd queue:
            # 32 indirect SWDGE gathers, then the batched dot tree.
            # Nothing here touches the DVE/ScalarE queues.
            off_sb = cpool.tile([128, mt], I32, tag="off")
            nc.gpsimd.dma_start(off_sb[:, :], off_d[:, :])
            wy_all = cpool.tile([128, mt, d], BF16, tag="wy_all")
            if not K_NOGATHER:
                for m in range(mt):
                    nc.gpsimd.indirect_dma_start(
                        out=wy_all[:, m, :], out_offset=None, in_=wn_d[:, :],
                        in_offset=bass.IndirectOffsetOnAxis(
                            ap=off_sb[:, m : m + 1], axis=0),
                        bounds_check=csh, oob_is_err=False)

            # gpsimd has no free-axis reduce, so the row dots are a
            # batched binary tree fold (bf16 products -> f32 halves ->
            # f32 folds), never touching the DVE/ScalarE queues.
            tz = cpool.tile([128, mt], F32, tag="tz")
            if K_NODOT:
                nc.vector.memset(tz[:, :], 0.0)
            else:
                pr_all = cpool.tile([128, mt, d], BF16, tag="pr_all")
                for m in range(mt):
                    nc.gpsimd.tensor_tensor(
                        out=pr_all[:, m, :], in0=xr_all[:, m, :],
                        in1=wy_all[:, m, :], op=OP.mult)
                prh = cpool.tile([128, mt, d // 2], F32, tag="prh")
                nc.gpsimd.tensor_tensor(
                    out=prh[:, :, :], in0=pr_all[:, :, 0 : d // 2],
                    in1=pr_all[:, :, d // 2 : d], op=OP.add)
                w = d // 4
                while w >= 1:
                    nc.gpsimd.tensor_tensor(
                        out=prh[:, :, 0:w], in0=prh[:, :, 0:w],
                        in1=prh[:, :, w : 2 * w], op=OP.add)
                    w //= 2
                nc.gpsimd.tensor_copy(out=tz[:, :], in_=prh[:, :, 0])

            # per-engine accumulators (separate tiles so ScalarE accum
            # writes and DVE reduce writes never WAW-serialize)
            accS = cpool.tile([128, mt, NNT], F32, tag="accS")
            accD = cpool.tile([128, mt, NNT], F32, tag="accD")

            lnc = cpool.tile([128, 1], F32, tag="lnc")
            nc.vector.memset(
                lnc[:, :], float(math.log(1.0 - math.exp(-S_SCALE * MARGIN))))
            csm = cpool.tile([128, 1], F32, tag="csm")
            nc.vector.memset(csm[:, :], float(S_SCALE * MARGIN))

            # main loop: fp8 DoubleRow matmuls into per-engine psum tiles
            # (psS: ScalarE table Exp w/ free accum row-sum; psD: DVE
            # Schraudolph exp2 via i32 affine + bitcast + reduce).
            NA = 5  # ntiles in collective phase A (classes [0, 9216))
            rr = cpool.tile([128, 2 * mt], F32, tag="rr")
            Bs = cpool.tile([128, 2], F32, tag="Bs")
            denA = cpool.tile([128, mt], F32, tag="denA")
            for ni, (n0, nw, scw) in enumerate(NTL):
                wt = wts[ni]
                dvw = nw - scw
                # chunk so no matmul write crosses a PSUM bank boundary
                # (psS/psD tiles are bank-aligned; scw itself need not
                # be a multiple of 512)
                nsub = [(j0, min(MMW, scw - j0)) for j0 in range(0, scw, MMW)]
                nsub += [(scw + j0, min(MMW, dvw - j0))
                         for j0 in range(0, dvw, MMW)]
                for m in range(mt):
                    psS = ppool.tile([128, 1536], F32, tag="psS", name="psS")
                    psD = ppool.tile([128, 512], F32, tag="psD", name="psD")
                    for k2 in range(kt2):
                        for j0, jw in nsub:
                            if j0 < scw:
                                dst = psS[:, j0 : j0 + jw]
                            else:
                                dst = psD[:, j0 - scw : j0 - scw + jw]
                            nc.tensor.matmul(
                                out=dst,
                                lhsT=xts[:, k2, :, m * 128 : (m + 1) * 128],
                                rhs=wt[:, k2, :, j0 : j0 + jw],
                                start=(k2 == 0), stop=(k2 == kt2 - 1),
                                perf_mode=mybir.MatmulPerfMode.DoubleRow)
                    # ScalarE share: exp -> bf16 with free row-sum accum
                    et = epool.tile([128, 1536], BF16, tag="et", name="et")
                    nc.scalar.activation(
                        et[:, :scw], psS[:, :scw], AF.Exp, bias=0.0, scale=KEXP,
                        accum_out=accS[:, m, ni : ni + 1])
                    # DVE share: Schraudolph exp2 (i16 convert + bf16
                    # bitcast; the reduce runs in DVE 2x mode)
                    ti = ipool.tile([128, 512], I16, tag="ti", name="ti")
                    nc.vector.tensor_scalar(
                        out=ti[:, :dvw], in0=psD[:, :dvw],
                        scalar1=float(A_SCH), scalar2=float(B_SCH),
                        op0=OP.mult, op1=OP.add)
                    nc.vector.tensor_reduce(
                        out=accD[:, m, ni : ni + 1],
                        in_=ti[:, :dvw].bitcast(BF16), axis=AX.X, op=OP.add)
                if ni == NA - 1:
                    # phase A: expsum partials for classes [0, 9216) + all
                    # target-logit partials; AllReduce overlaps ni 5-6.
                    ccsbA = cpool.tile([128, 2 * mt], F32, tag="ccsbA")
                    tmpA = cpool.tile([128, mt], F32, tag="tmpA")
                    nc.vector.tensor_reduce(
                        out=ccsbA[:, 0:mt], in_=accS[:, :, 0:NA],
                        axis=AX.X, op=OP.add)
                    nc.vector.tensor_reduce(
                        out=tmpA[:, :], in_=accD[:, :, 0:NA], axis=AX.X, op=OP.add)
                    nc.vector.tensor_tensor(
                        out=ccsbA[:, 0:mt], in0=ccsbA[:, 0:mt], in1=tmpA[:, :],
                        op=OP.add)
                    nc.vector.tensor_copy(out=ccsbA[:, mt : 2 * mt], in_=tz[:, :])
                    nc.sync.dma_start(ccA_in[:, :], ccsbA[:, :])
                    nc.gpsimd.collective_compute(
                        "AllReduce", OP.add,
                        replica_groups=[list(range(cores))],
                        ins=[ccA_in.ap().opt()], outs=[ccA_out.ap().opt()])
                if ni == NA:
                    # phase-A result readback (sync queue; waits on the
                    # collective, but nothing downstream consumes it
                    # until the tail, so no engine queue blocks on it)
                    nc.sync.dma_start(rr[:, :], ccA_out[:, :])

            # phase B: expsum partials for classes [9216, 12544)
            ccsbB = cpool.tile([128, mt], F32, tag="ccsbB")
            tmpB = cpool.tile([128, mt], F32, tag="tmpB")
            nc.vector.tensor_reduce(
                out=ccsbB[:, 0:mt], in_=accS[:, :, NA:NNT], axis=AX.X, op=OP.add)
            nc.vector.tensor_reduce(
                out=tmpB[:, :], in_=accD[:, :, NA:NNT], axis=AX.X, op=OP.add)
            nc.vector.tensor_tensor(
                out=ccsbB[:, 0:mt], in0=ccsbB[:, 0:mt], in1=tmpB[:, :], op=OP.add)
            nc.sync.dma_start(ccB_in[:, :], ccsbB[:, :])
            nc.gpsimd.collective_compute(
                "AllReduce", OP.add, replica_groups=[list(range(cores))],
                ins=[ccB_in.ap().opt()], outs=[ccB_out.ap().opt()])

            # phase-A epilogue (emitted at the tail so it can't
            # head-of-line block the Scalar/Vector queues behind the
            # collective): B = s*t_tot; Bs0 = sum(B);
            # Es = exp(B + ln(1-e^{-sm})); denA = expsumA - Es - pads
            B = cpool.tile([128, mt], F32, tag="B")
            nc.vector.tensor_scalar(
                out=B[:, :], in0=rr[:, mt : 2 * mt], scalar1=S_SCALE,
                scalar2=None, op0=OP.mult, op1=OP.add, accum_out=Bs[:, 0:1])
            Es = cpool.tile([128, mt], F32, tag="Es")
            nc.scalar.activation(
                Es[:, :], B[:, :], AF.Exp, bias=lnc[:, :1], scale=1.0)
            nc.vector.tensor_tensor(
                out=denA[:, :], in0=rr[:, 0:mt], in1=Es[:, :], op=OP.subtract)
            nc.vector.tensor_scalar(
                out=denA[:, :], in0=denA[:, :], scalar1=float(C_PAD_TOTAL),
                scalar2=None, op0=OP.subtract)

            rrB = cpool.tile([128, mt], F32, tag="rrB")
            nc.sync.dma_start(rrB[:, :], ccB_out[:, :])

            # exposed tail: den = denA + expsumB; loss = s*m - (sum(B) -
            # sum(ln den))/n
            den = cpool.tile([128, mt], F32, tag="den")
            nc.vector.tensor_tensor(
                out=den[:, :], in0=denA[:, :], in1=rrB[:, :], op=OP.add)
            lden = cpool.tile([128, mt], F32, tag="lden")
            nc.scalar.activation(
                lden[:, :], den[:, :], AF.Ln, accum_out=Bs[:, 1:2])
            diff = cpool.tile([128, 1], F32, tag="diff")
            nc.vector.tensor_tensor(
                out=diff[:, :], in0=Bs[:, 0:1], in1=Bs[:, 1:2], op=OP.subtract)
            zs = cpool.tile([128, 1], F32, tag="zs")
            nc.gpsimd.partition_all_reduce(zs[:, :], diff[:, :], 128, ReduceOp.add)
            res = cpool.tile([1, 1], F32, tag="res")
            nc.scalar.activation(
                res[:, :], zs[:1, :], AF.Identity,
                bias=csm[:1, :], scale=-1.0 / n)
            nc.sync.dma_start(out_d[:, :], res[:, :])
    nc.compile()
    return nc


def shard_inputs(x, labels, W, n=N, d=D, csh=CSH, cores=CORES):
    x32 = np.ascontiguousarray(np.asarray(x), dtype=np.float32)
    xn = x32 / np.sqrt((x32 * x32).sum(axis=1, keepdims=True))
    # [ki, k2, ko, n] fp8 DoubleRow layout: d = k2*256 + ko*128 + ki
    xq = (xn.T * XSCALE).reshape(2, 2, 128, n).transpose(2, 0, 1, 3)
    xq = np.ascontiguousarray(np.clip(xq, -240, 240)).astype(ml_dtypes.float8_e4m3)
    # [p, m, d] bf16 rows
    xnb = np.ascontiguousarray(
        xn.reshape(n // 128, 128, d).transpose(1, 0, 2)).astype(ml_dtypes.bfloat16)
    lab = np.asarray(labels).astype(np.int64).reshape(n)
    in_maps = []
    for r in range(cores):
        Wc = np.ascontiguousarray(np.asarray(W)[r * csh : (r + 1) * csh], dtype=np.float32)
        Wp = np.zeros((CSHP, d), np.float32)
        Wp[:csh] = Wc
        wq = (Wp.T * WSCALE).reshape(2, 2, 128, CSHP).transpose(2, 0, 1, 3)
        wq = np.ascontiguousarray(np.clip(wq, -240, 240)).astype(ml_dtypes.float8_e4m3)
        loc = lab - r * csh
        off = np.where((loc >= 0) & (loc < csh), loc, csh).astype(np.int32)
        off = np.ascontiguousarray(off.reshape(n // 128, 128).T)
        wn = np.vstack([Wc, np.zeros((1, d), np.float32)]).astype(ml_dtypes.bfloat16)
        in_maps.append({
            "xq": xq, "xn": xnb, "wq": wq,
            "wn": np.ascontiguousarray(wn), "off": off,
        })
    return in_maps


_CACHE = {}


def kernel(x, labels, W, **run_kwargs):
    if "nc" not in _CACHE:
        _CACHE["nc"] = build()
    nc = _CACHE["nc"]
    in_maps = shard_inputs(x, labels, W)
    res = run_bass_kernel_spmd(nc, in_maps, core_ids=list(range(CORES)), **run_kwargs)
    out = np.asarray(res.results[0]["out"], dtype=np.float32).reshape(())
    if run_kwargs:
        return out, res
    return out


# revision 34
# speedup vs baseline: 1.2272x; 1.0008x over previous
"""AdMSoftmaxLoss on 8 TRN2 NeuronCores (Bass/Tile).

Math (matches the reference exactly):
    xn    = x / ||x||_row
    wf    = xn @ W.T                      [N, C]
    t_i   = wf[i, y_i]
    num_i = s*(t_i - m)
    den_i = exp(num_i) + sum_j exp(s*wf_ij) - exp(s*t_i)
    loss  = -mean(num_i - log(den_i))

Distribution: vocab/tensor parallel. W's class dim is sharded 8 ways;
each core computes its slice of the logits as an fp8e4 DoubleRow matmul.
x is row-normalized on the host, so the exp scale is a compile-time
constant and PSUM drains have no data dependency on a norms pass.

v2 structure (from trace analysis of v1):
- PSUM per (ni, m) tile is split into two pool tiles, psS (<=3 banks,
  ScalarE table-Exp drain w/ accum_out row-sums) and psD (1 bank, DVE
  Schraudolph exp2 drain). With one shared tile the framework chained
  DVE's psum read behind ScalarE's accumulator drain, making the drain
  chain ~3.2us > the 2.1us GPIO-throttled matmul fill and costing ~1us
  every other tile.
- The target-dot products are single gpsimd scalar_tensor_tensor
  instructions (mult, mult, accum_out=tz). v1 put the dot row-sum
  reduces on DVE, where the first one head-of-line blocked the in-order
  DVE queue for 38us waiting on the SWDGE gathers.
- Class dim padded 12500 -> 12544 (44 pad cols, v1 used 300).
- First W tile is 1024 cols so the matmul ramp starts earlier; xq is
  split across the scalar+vector DMA queues.
- Phase A AllReduce (class tiles 0-4 + target partials) overlaps tiles
  5-6; its epilogue math is precomputed mid-body so the exposed tail is
  only: last drain -> AllReduce B -> den add/log -> output.
"""

import math

import ml_dtypes
import numpy as np

import concourse.bacc as bacc
import concourse.bass_isa as bass_isa
import concourse.bass as bass
import concourse.mybir as mybir
import concourse.tile as tile
from concourse.bass_utils import run_bass_kernel_spmd

N, D, C, CORES = 4096, 512, 100000, 8
CSH = C // CORES
S_SCALE, MARGIN = 30.0, 0.4

F32 = mybir.dt.float32
BF16 = mybir.dt.bfloat16
I32 = mybir.dt.int32
I16 = mybir.dt.int16
FP8 = mybir.dt.float8e4
AF = mybir.ActivationFunctionType
OP = mybir.AluOpType
AX = mybir.AxisListType
ReduceOp = bass_isa.ReduceOp

XSCALE, WSCALE = 360.0, 256.0
KEXP = S_SCALE / (XSCALE * WSCALE)          # exp scale on raw psum values
# bf16 Schraudolph: i16 codes (top 16 bits of the f32 trick) -> bitcast
# bf16. Halves the DVE reduce cost (2x-mode bf16 TENSOR_REDUCE); the
# coarser exp approximation is zero-mean and covers only ~28% of the
# expsum, noise-level for the final loss.
A_SCH = (1 << 7) * math.log2(math.e) * KEXP   # Schraudolph multiplier
B_SCH = 1.0648707e9 / 65536.0                 # tuned magic constant

# Per-core class dim padded 12500 -> 12544 with zero-weight columns.
# NTL entries: (start, width, ScalarE share width). ScalarE share is a
# multiple of 512 so the psS/psD pool tiles stay PSUM-bank aligned.
CSHP = 12544
# 12544 = 7 uniform tiles of 1792: ScalarE drains 1280 cols/tile (80%
# of the 1.84us GPIO-throttled fill), DVE 512 (74%) -- both engines
# keep slack so psum WAR never stalls the matmul stream.
NTL = [(i * 1792, 1792, 1280) for i in range(7)]
assert NTL[-1][0] + NTL[-1][1] == CSHP
NNT = len(NTL)
MMW = 512  # matmul moving width (ISA limit: 512 psum cols)
# pad cols 12500..12544 sit in the last tile's DVE share (global
# [12032,12544)): each contributes the exactly computable Schraudolph
# constant for input 0.
_C0 = float(np.array(int(np.float32(0.0) * np.float32(A_SCH)
                         + np.float32(B_SCH)),
                     np.int16).view(ml_dtypes.bfloat16))
C_PAD_TOTAL = CORES * 44 * _C0

import os
K_NODOT = bool(int(os.environ.get("K_NODOT", "0")))
K_NOGATHER = bool(int(os.environ.get("K_NOGATHER", "0")))


def build(n=N, d=D, csh=CSH, cores=CORES):
    mt, kt2 = n // 128, d // 256
    nc = bacc.Bacc("TRN2", target_bir_lowering=False, debug=False, num_devices=cores)

    # host-packed DoubleRow layouts: [ki, k2, ko, *] with d = k2*256 + ko*128 + ki
    xq_d = nc.dram_tensor("xq", [128, kt2, 2, n], FP8, kind="ExternalInput")
    xn_d = nc.dram_tensor("xn", [128, mt, d], BF16, kind="ExternalInput")
    wq_d = nc.dram_tensor("wq", [128, kt2, 2, CSHP], FP8, kind="ExternalInput")
    # one extra all-zeros row: out-of-shard labels gather it (no memsets)
    wn_d = nc.dram_tensor("wn", [csh + 1, d], BF16, kind="ExternalInput")
    off_d = nc.dram_tensor("off", [128, mt], I32, kind="ExternalInput")
    out_d = nc.dram_tensor("out", [1, 1], F32, kind="ExternalOutput")
    ccA_in = nc.dram_tensor("ccA_in", [128, 2 * mt], F32)
    ccA_out = nc.dram_tensor("ccA_out", [128, 2 * mt], F32, addr_space="Shared")
    ccB_in = nc.dram_tensor("ccB_in", [128, mt], F32)
    ccB_out = nc.dram_tensor("ccB_out", [128, mt], F32, addr_space="Shared")


    with tile.TileContext(nc) as tc:
        with (
            tc.tile_pool(name="const", bufs=1) as cpool,
            tc.tile_pool(name="wstream", bufs=4) as wpool,
            tc.tile_pool(name="escr", bufs=3) as epool,
            tc.tile_pool(name="iscr", bufs=3) as ipool,
            tc.tile_pool(name="dscr", bufs=2) as dpool,
            tc.tile_pool(name="psum", bufs=2, space="PSUM") as ppool,
        ):
            # stationary x^T (fp8 DoubleRow) on the scalar DMA queue,
            # k2=0 half first so the ramp's k2=0 matmuls start earlier
            xts = cpool.tile([128, kt2, 2, n], FP8, tag="xts")
            nc.scalar.dma_start(xts[:, 0, :, :], xq_d[:, 0, :, :])
            nc.scalar.dma_start(xts[:, 1, :, :], xq_d[:, 1, :, :])
            xr_all = cpool.tile([128, mt, d], BF16, tag="xr_all")
            # W stream + xn ride the sync queue; xn after the first two W
            # tiles so the matmul ramp isn't starved.
            wts = []
            for ni, (n0, nw, _) in enumerate(NTL):
                wt = wpool.tile([128, kt2, 2, 2048], FP8, tag="wt", name="wt")
                if ni == 0:
                    nc.sync.dma_start(wt[:, 0, :, :nw], wq_d[:, 0, :, n0 : n0 + nw])
                    nc.sync.dma_start(wt[:, 1, :, :nw], wq_d[:, 1, :, n0 : n0 + nw])
                else:
                    nc.sync.dma_start(wt[:, :, :, :nw], wq_d[:, :, :, n0 : n0 + nw])
                wts.append(wt)
                if ni == 1:
                    nc.sync.dma_start(xr_all[:, :, :], xn_d[:, :, :])

            # target-row gathers + dot products, all on the gpsimd queue:
            # 32 indirect SWDGE gathers, then the batched dot tree.
            # Nothing here touches the DVE/ScalarE queues.
            off_sb = cpool.tile([128, mt], I32, tag="off")
            nc.gpsimd.dma_start(off_sb[:, :], off_d[:, :])
            wy_all = cpool.tile([128, mt, d], BF16, tag="wy_all")
            if not K_NOGATHER:
                for m in range(mt):
                    nc.gpsimd.indirect_dma_start(
                        out=wy_all[:, m, :], out_offset=None, in_=wn_d[:, :],
                        in_offset=bass.IndirectOffsetOnAxis(
                            ap=off_sb[:, m : m + 1], axis=0),
                        bounds_check=csh, oob_is_err=False)

            # gpsimd has no free-axis reduce, so the row dots are a
            # batched binary tree fold (bf16 products -> f32 halves ->
            # f32 folds), never touching the DVE/ScalarE queues.
            tz = cpool.tile([128, mt], F32, tag="tz")
            if K_NODOT:
                nc.vector.memset(tz[:, :], 0.0)
            else:
                pr_all = cpool.tile([128, mt, d], BF16, tag="pr_all")
                for m in range(mt):
                    nc.gpsimd.tensor_tensor(
                        out=pr_all[:, m, :], in0=xr_all[:, m, :],
                        in1=wy_all[:, m, :], op=OP.mult)
                prh = cpool.tile([128, mt, d // 2], F32, tag="prh")
                nc.gpsimd.tensor_tensor(
                    out=prh[:, :, :], in0=pr_all[:, :, 0 : d // 2],
                    in1=pr_all[:, :, d // 2 : d], op=OP.add)
                w = d // 4
                while w >= 1:
                    nc.gpsimd.tensor_tensor(
                        out=prh[:, :, 0:w], in0=prh[:, :, 0:w],
                        in1=prh[:, :, w : 2 * w], op=OP.add)
                    w //= 2
                nc.gpsimd.tensor_copy(out=tz[:, :], in_=prh[:, :, 0])

            # per-engine accumulators (separate tiles so ScalarE accum
            # writes and DVE reduce writes never WAW-serialize)
            accS = cpool.tile([128, mt, NNT], F32, tag="accS")
            accD = cpool.tile([128, mt, NNT], F32, tag="accD")

            lnc = cpool.tile([128, 1], F32, tag="lnc")
            nc.vector.memset(
                lnc[:, :], float(math.log(1.0 - math.exp(-S_SCALE * MARGIN))))
            csm = cpool.tile([128, 1], F32, tag="csm")
            nc.vector.memset(csm[:, :], float(S_SCALE * MARGIN))

            # main loop: fp8 DoubleRow matmuls into per-engine psum tiles
            # (psS: ScalarE table Exp w/ free accum row-sum; psD: DVE
            # Schraudolph exp2 via i32 affine + bitcast + reduce).
            NA = 5  # ntiles in collective phase A (classes [0, 9216))
            rr = cpool.tile([128, 2 * mt], F32, tag="rr")
            Bs = cpool.tile([128, 2], F32, tag="Bs")
            denA = cpool.tile([128, mt], F32, tag="denA")
            for ni, (n0, nw, scw) in enumerate(NTL):
                wt = wts[ni]
                dvw = nw - scw
                # chunk so no matmul write crosses a PSUM bank boundary
                # (psS/psD tiles are bank-aligned; scw itself need not
                # be a multiple of 512)
                nsub = [(j0, min(MMW, scw - j0)) for j0 in range(0, scw, MMW)]
                nsub += [(scw + j0, min(MMW, dvw - j0))
                         for j0 in range(0, dvw, MMW)]
                for m in range(mt):
                    psS = ppool.tile([128, 1536], F32, tag="psS", name="psS")
                    psD = ppool.tile([128, 512], F32, tag="psD", name="psD")
                    for k2 in range(kt2):
                        for j0, jw in nsub:
                            if j0 < scw:
                                dst = psS[:, j0 : j0 + jw]
                            else:
                                dst = psD[:, j0 - scw : j0 - scw + jw]
                            nc.tensor.matmul(
                                out=dst,
                                lhsT=xts[:, k2, :, m * 128 : (m + 1) * 128],
                                rhs=wt[:, k2, :, j0 : j0 + jw],
                                start=(k2 == 0), stop=(k2 == kt2 - 1),
                                perf_mode=mybir.MatmulPerfMode.DoubleRow)
                    # ScalarE share: exp -> bf16 with free row-sum accum
                    et = epool.tile([128, 1536], BF16, tag="et", name="et")
                    nc.scalar.activation(
                        et[:, :scw], psS[:, :scw], AF.Exp, bias=0.0, scale=KEXP,
                        accum_out=accS[:, m, ni : ni + 1])
                    # DVE share: Schraudolph exp2 (i16 convert + bf16
                    # bitcast; the reduce runs in DVE 2x mode)
                    ti = ipool.tile([128, 512], I16, tag="ti", name="ti")
                    nc.vector.tensor_scalar(
                        out=ti[:, :dvw], in0=psD[:, :dvw],
                        scalar1=float(A_SCH), scalar2=float(B_SCH),
                        op0=OP.mult, op1=OP.add)
                    nc.vector.tensor_reduce(
                        out=accD[:, m, ni : ni + 1],
                        in_=ti[:, :dvw].bitcast(BF16), axis=AX.X, op=OP.add)
                if ni == NA - 1:
                    # phase A: expsum partials for classes [0, 9216) + all
                    # target-logit partials; AllReduce overlaps ni 5-6.
                    ccsbA = cpool.tile([128, 2 * mt], F32, tag="ccsbA")
                    tmpA = cpool.tile([128, mt], F32, tag="tmpA")
                    nc.vector.tensor_reduce(
                        out=ccsbA[:, 0:mt], in_=accS[:, :, 0:NA],
                        axis=AX.X, op=OP.add)
                    nc.vector.tensor_reduce(
                        out=tmpA[:, :], in_=accD[:, :, 0:NA], axis=AX.X, op=OP.add)
                    nc.vector.tensor_tensor(
                        out=ccsbA[:, 0:mt], in0=ccsbA[:, 0:mt], in1=tmpA[:, :],
                        op=OP.add)
                    nc.vector.tensor_copy(out=ccsbA[:, mt : 2 * mt], in_=tz[:, :])
                    nc.sync.dma_start(ccA_in[:, :], ccsbA[:, :])
                    nc.gpsimd.collective_compute(
                        "AllReduce", OP.add,
                        replica_groups=[list(range(cores))],
                        ins=[ccA_in.ap().opt()], outs=[ccA_out.ap().opt()])
                if ni == NA:
                    # phase-A result readback (sync queue; waits on the
                    # collective, but nothing downstream consumes it
                    # until the tail, so no engine queue blocks on it)
                    nc.sync.dma_start(rr[:, :], ccA_out[:, :])

            # phase B: expsum partials for classes [9216, 12544)
            ccsbB = cpool.tile([128, mt], F32, tag="ccsbB")
            tmpB = cpool.tile([128, mt], F32, tag="tmpB")
            nc.vector.tensor_reduce(
                out=ccsbB[:, 0:mt], in_=accS[:, :, NA:NNT], axis=AX.X, op=OP.add)
            nc.vector.tensor_reduce(
                out=tmpB[:, :], in_=accD[:, :, NA:NNT], axis=AX.X, op=OP.add)
            nc.vector.tensor_tensor(
                out=ccsbB[:, 0:mt], in0=ccsbB[:, 0:mt], in1=tmpB[:, :], op=OP.add)
            nc.sync.dma_start(ccB_in[:, :], ccsbB[:, :])
            nc.gpsimd.collective_compute(
                "AllReduce", OP.add, replica_groups=[list(range(cores))],
                ins=[ccB_in.ap().opt()], outs=[ccB_out.ap().opt()])

            # phase-A epilogue (emitted at the tail so it can't
            # head-of-line block the Scalar/Vector queues behind the
            # collective): B = s*t_tot; Bs0 = sum(B);
            # Es = exp(B + ln(1-e^{-sm})); denA = expsumA - Es - pads
            B = cpool.tile([128, mt], F32, tag="B")
            nc.vector.tensor_scalar(
                out=B[:, :], in0=rr[:, mt : 2 * mt], scalar1=S_SCALE,
                scalar2=None, op0=OP.mult, op1=OP.add, accum_out=Bs[:, 0:1])
            Es = cpool.tile([128, mt], F32, tag="Es")
            nc.scalar.activation(
                Es[:, :], B[:, :], AF.Exp, bias=lnc[:, :1], scale=1.0)
            nc.vector.tensor_tensor(
                out=denA[:, :], in0=rr[:, 0:mt], in1=Es[:, :], op=OP.subtract)
            nc.vector.tensor_scalar(
                out=denA[:, :], in0=denA[:, :], scalar1=float(C_PAD_TOTAL),
                scalar2=None, op0=OP.subtract)

            rrB = cpool.tile([128, mt], F32, tag="rrB")
            nc.sync.dma_start(rrB[:, :], ccB_out[:, :])

            # exposed tail: den = denA + expsumB; loss = s*m - (sum(B) -
            # sum(ln den))/n
            den = cpool.tile([128, mt], F32, tag="den")
            nc.vector.tensor_tensor(
                out=den[:, :], in0=denA[:, :], in1=rrB[:, :], op=OP.add)
            lden = cpool.tile([128, mt], F32, tag="lden")
            nc.scalar.activation(
                lden[:, :], den[:, :], AF.Ln, accum_out=Bs[:, 1:2])
            diff = cpool.tile([128, 1], F32, tag="diff")
            nc.vector.tensor_tensor(
                out=diff[:, :], in0=Bs[:, 0:1], in1=Bs[:, 1:2], op=OP.subtract)
            zs = cpool.tile([128, 1], F32, tag="zs")
            nc.gpsimd.partition_all_reduce(zs[:, :], diff[:, :], 128, ReduceOp.add)
            res = cpool.tile([1, 1], F32, tag="res")
            nc.scalar.activation(
                res[:, :], zs[:1, :], AF.Identity,
                bias=csm[:1, :], scale=-1.0 / n)
            nc.sync.dma_start(out_d[:, :], res[:, :])
    nc.compile()
    return nc


def shard_inputs(x, labels, W, n=N, d=D, csh=CSH, cores=CORES):
    x32 = np.ascontiguousarray(np.asarray(x), dtype=np.float32)
    xn = x32 / np.sqrt((x32 * x32).sum(axis=1, keepdims=True))
    # [ki, k2, ko, n] fp8 DoubleRow layout: d = k2*256 + ko*128 + ki
    xq = (xn.T * XSCALE).reshape(2, 2, 128, n).transpose(2, 0, 1, 3)
    xq = np.ascontiguousarray(np.clip(xq, -240, 240)).astype(ml_dtypes.float8_e4m3)
    # [p, m, d] bf16 rows
    xnb = np.ascontiguousarray(
        xn.reshape(n // 128, 128, d).transpose(1, 0, 2)).astype(ml_dtypes.bfloat16)
    lab = np.asarray(labels).astype(np.int64).reshape(n)
    in_maps = []
    for r in range(cores):
        Wc = np.ascontiguousarray(np.asarray(W)[r * csh : (r + 1) * csh], dtype=np.float32)
        Wp = np.zeros((CSHP, d), np.float32)
        Wp[:csh] = Wc
        wq = (Wp.T * WSCALE).reshape(2, 2, 128, CSHP).transpose(2, 0, 1, 3)
        wq = np.ascontiguousarray(np.clip(wq, -240, 240)).astype(ml_dtypes.float8_e4m3)
        loc = lab - r * csh
        off = np.where((loc >= 0) & (loc < csh), loc, csh).astype(np.int32)
        off = np.ascontiguousarray(off.reshape(n // 128, 128).T)
        wn = np.vstack([Wc, np.zeros((1, d), np.float32)]).astype(ml_dtypes.bfloat16)
        in_maps.append({
            "xq": xq, "xn": xnb, "wq": wq,
            "wn": np.ascontiguousarray(wn), "off": off,
        })
    return in_maps


_CACHE = {}


def kernel(x, labels, W, **run_kwargs):
    if "nc" not in _CACHE:
        _CACHE["nc"] = build()
    nc = _CACHE["nc"]
    in_maps = shard_inputs(x, labels, W)
    res = run_bass_kernel_spmd(nc, in_maps, core_ids=list(range(CORES)), **run_kwargs)
    out = np.asarray(res.results[0]["out"], dtype=np.float32).reshape(())
    if run_kwargs:
        return out, res
    return out


# revision 36
# speedup vs baseline: 1.2646x; 1.0305x over previous
"""AdMSoftmaxLoss on 8 TRN2 NeuronCores (Bass/Tile).

Math (matches the reference exactly):
    xn    = x / ||x||_row
    wf    = xn @ W.T                      [N, C]
    t_i   = wf[i, y_i]
    num_i = s*(t_i - m)
    den_i = exp(num_i) + sum_j exp(s*wf_ij) - exp(s*t_i)
    loss  = -mean(num_i - log(den_i))

Distribution: vocab/tensor parallel. W's class dim is sharded 8 ways;
each core computes its slice of the logits as an fp8e4 DoubleRow matmul.
x is row-normalized on the host, so the exp scale is a compile-time
constant and PSUM drains have no data dependency on a norms pass.

Structure (from trace analysis; ~466-487us vs the 646us baseline):
- Seven uniform class tiles of 1792 (12544 = 7*1792, 44 pad cols).
  PSUM per (ni, m) tile is split into per-engine pool tiles: psS
  (3 banks, ScalarE table-Exp drain over 1280 cols w/ accum_out
  row-sums) and psD (1 bank, DVE Schraudolph-exp drain over 512). A
  single shared psum tile made the framework chain DVE's read behind
  ScalarE's accumulator drain (~1us stall every other tile); the split
  keeps both drains at ~80% of the 263ns/MM GPIO-throttled fill.
- DVE Schraudolph uses i16 codes bitcast to bf16 (top 16 bits of the
  classic f32 trick): half the SBUF traffic, zero-mean approximation
  error on ~28% of the expsum, noise-level for the final loss.
- The target-row gather (32 indirect SWDGE DMAs), the per-row dot
  products (gpsimd tensor_tensor), and their d-axis reduction (a
  batched binary tree fold, bf16 -> f32) all stay on the in-order
  gpsimd queue: any DVE/ScalarE consumer of this slow chain gets
  scheduled early in those queues and head-of-line blocks the psum
  drains (v1 lost 47us + a K=4/8 HAM cascade to exactly that).
- Matmul chunking only avoids PSUM bank crossings (512-col chunks
  within psS/psD), so the ScalarE/DVE drain split does not need to be
  a multiple of 512.
- Phase A AllReduce (class tiles 0-4 + all target partials) launches
  at ~75% of the body and overlaps tiles 5-6. Its dependent epilogue
  math is emitted at the tail (NOT mid-body: a mid-body ScalarE op
  waiting on the collective head-of-line blocks the drain queue when
  the collective is slow, and the straggling core then stalls everyone
  in phase B). Exposed tail: last drain -> AllReduce B (expsums of
  tiles 5-6) -> den add/log/mean -> output DMA.
- xq and the first W tile are loaded in k2-halves so the ramp's k2=0
  matmuls start one half-transfer earlier.

Known non-fixes (measured): a start-of-kernel barrier collective
aligns cores and makes phases A/B cost ~2us, but the alignment stall
itself costs the full start skew plus a HAM K=4/8 re-warm penalty --
net worse than letting the overlapped phase A absorb skew. The
GPIO-2 throttle (k=13/16, 263ns/MM) comes and goes per run
independent of kernel structure; when absent, the drains (not the PE)
bind at ~1.9us/tile, which the 1280/512 split balances.
"""

import math

import ml_dtypes
import numpy as np

import concourse.bacc as bacc
import concourse.bass_isa as bass_isa
import concourse.bass as bass
import concourse.mybir as mybir
import concourse.tile as tile
from concourse.bass_utils import run_bass_kernel_spmd

N, D, C, CORES = 4096, 512, 100000, 8
CSH = C // CORES
S_SCALE, MARGIN = 30.0, 0.4

F32 = mybir.dt.float32
BF16 = mybir.dt.bfloat16
I32 = mybir.dt.int32
I16 = mybir.dt.int16
FP8 = mybir.dt.float8e4
AF = mybir.ActivationFunctionType
OP = mybir.AluOpType
AX = mybir.AxisListType
ReduceOp = bass_isa.ReduceOp

XSCALE, WSCALE = 360.0, 256.0
KEXP = S_SCALE / (XSCALE * WSCALE)          # exp scale on raw psum values
# bf16 Schraudolph: i16 codes (top 16 bits of the f32 trick) -> bitcast
# bf16. Halves the DVE reduce cost (2x-mode bf16 TENSOR_REDUCE); the
# coarser exp approximation is zero-mean and covers only ~28% of the
# expsum, noise-level for the final loss.
A_SCH = (1 << 7) * math.log2(math.e) * KEXP   # Schraudolph multiplier
B_SCH = 1.0648707e9 / 65536.0                 # tuned magic constant

# Per-core class dim padded 12500 -> 12544 with zero-weight columns.
# NTL entries: (start, width, ScalarE share width). ScalarE share is a
# multiple of 512 so the psS/psD pool tiles stay PSUM-bank aligned.
CSHP = 12544
# 12544 = 7 uniform tiles of 1792: ScalarE drains 1280 cols/tile (80%
# of the 1.84us GPIO-throttled fill), DVE 512 (74%) -- both engines
# keep slack so psum WAR never stalls the matmul stream.
NTL = [(i * 1792, 1792, 1280) for i in range(7)]
assert NTL[-1][0] + NTL[-1][1] == CSHP
NNT = len(NTL)
MMW = 512  # matmul moving width (ISA limit: 512 psum cols)
# pad cols 12500..12544 sit in the last tile's DVE share (global
# [12032,12544)): each contributes the exactly computable Schraudolph
# constant for input 0.
_C0 = float(np.array(int(np.float32(0.0) * np.float32(A_SCH)
                         + np.float32(B_SCH)),
                     np.int16).view(ml_dtypes.bfloat16))
C_PAD_TOTAL = CORES * 44 * _C0

import os
K_NODOT = bool(int(os.environ.get("K_NODOT", "0")))
K_NOGATHER = bool(int(os.environ.get("K_NOGATHER", "0")))


def build(n=N, d=D, csh=CSH, cores=CORES):
    mt, kt2 = n // 128, d // 256
    nc = bacc.Bacc("TRN2", target_bir_lowering=False, debug=False, num_devices=cores)

    # host-packed DoubleRow layouts: [ki, k2, ko, *] with d = k2*256 + ko*128 + ki
    xq_d = nc.dram_tensor("xq", [128, kt2, 2, n], FP8, kind="ExternalInput")
    xn_d = nc.dram_tensor("xn", [128, mt, d], BF16, kind="ExternalInput")
    wq_d = nc.dram_tensor("wq", [128, kt2, 2, CSHP], FP8, kind="ExternalInput")
    # one extra all-zeros row: out-of-shard labels gather it (no memsets)
    wn_d = nc.dram_tensor("wn", [csh + 1, d], BF16, kind="ExternalInput")
    off_d = nc.dram_tensor("off", [128, mt], I32, kind="ExternalInput")
    out_d = nc.dram_tensor("out", [1, 1], F32, kind="ExternalOutput")
    ccA_in = nc.dram_tensor("ccA_in", [128, 2 * mt], F32)
    ccA_out = nc.dram_tensor("ccA_out", [128, 2 * mt], F32, addr_space="Shared")
    ccB_in = nc.dram_tensor("ccB_in", [128, mt], F32)
    ccB_out = nc.dram_tensor("ccB_out", [128, mt], F32, addr_space="Shared")


    with tile.TileContext(nc) as tc:
        with (
            tc.tile_pool(name="const", bufs=1) as cpool,
            tc.tile_pool(name="wstream", bufs=4) as wpool,
            tc.tile_pool(name="escr", bufs=3) as epool,
            tc.tile_pool(name="iscr", bufs=3) as ipool,
            tc.tile_pool(name="psum", bufs=2, space="PSUM") as ppool,
        ):
            # stationary x^T (fp8 DoubleRow) on the scalar DMA queue,
            # k2=0 half first so the ramp's k2=0 matmuls start earlier
            xts = cpool.tile([128, kt2, 2, n], FP8, tag="xts")
            nc.scalar.dma_start(xts[:, 0, :, :], xq_d[:, 0, :, :])
            nc.scalar.dma_start(xts[:, 1, :, :], xq_d[:, 1, :, :])
            xr_all = cpool.tile([128, mt, d], BF16, tag="xr_all")
            # W stream + xn ride the sync queue; xn after the first two W
            # tiles so the matmul ramp isn't starved.
            wts = []
            for ni, (n0, nw, _) in enumerate(NTL):
                wt = wpool.tile([128, kt2, 2, 2048], FP8, tag="wt", name="wt")
                if ni == 0:
                    nc.sync.dma_start(wt[:, 0, :, :nw], wq_d[:, 0, :, n0 : n0 + nw])
                    nc.sync.dma_start(wt[:, 1, :, :nw], wq_d[:, 1, :, n0 : n0 + nw])
                else:
                    nc.sync.dma_start(wt[:, :, :, :nw], wq_d[:, :, :, n0 : n0 + nw])
                wts.append(wt)
                if ni == 1:
                    nc.sync.dma_start(xr_all[:, :, :], xn_d[:, :, :])

            # target-row gathers + dot products, all on the gpsimd queue:
            # 32 indirect SWDGE gathers, then the batched dot tree.
            # Nothing here touches the DVE/ScalarE queues.
            off_sb = cpool.tile([128, mt], I32, tag="off")
            nc.gpsimd.dma_start(off_sb[:, :], off_d[:, :])
            wy_all = cpool.tile([128, mt, d], BF16, tag="wy_all")
            if not K_NOGATHER:
                for m in range(mt):
                    nc.gpsimd.indirect_dma_start(
                        out=wy_all[:, m, :], out_offset=None, in_=wn_d[:, :],
                        in_offset=bass.IndirectOffsetOnAxis(
                            ap=off_sb[:, m : m + 1], axis=0),
                        bounds_check=csh, oob_is_err=False)

            # gpsimd has no free-axis reduce, so the row dots are a
            # batched binary tree fold (bf16 products -> f32 halves ->
            # f32 folds), never touching the DVE/ScalarE queues.
            tz = cpool.tile([128, mt], F32, tag="tz")
            if K_NODOT:
                nc.vector.memset(tz[:, :], 0.0)
            else:
                pr_all = cpool.tile([128, mt, d], BF16, tag="pr_all")
                for m in range(mt):
                    nc.gpsimd.tensor_tensor(
                        out=pr_all[:, m, :], in0=xr_all[:, m, :],
                        in1=wy_all[:, m, :], op=OP.mult)
                prh = cpool.tile([128, mt, d // 2], F32, tag="prh")
                nc.gpsimd.tensor_tensor(
                    out=prh[:, :, :], in0=pr_all[:, :, 0 : d // 2],
                    in1=pr_all[:, :, d // 2 : d], op=OP.add)
                w = d // 4
                while w >= 1:
                    nc.gpsimd.tensor_tensor(
                        out=prh[:, :, 0:w], in0=prh[:, :, 0:w],
                        in1=prh[:, :, w : 2 * w], op=OP.add)
                    w //= 2
                nc.gpsimd.tensor_copy(out=tz[:, :], in_=prh[:, :, 0])

            # per-engine accumulators (separate tiles so ScalarE accum
            # writes and DVE reduce writes never WAW-serialize)
            accS = cpool.tile([128, mt, NNT], F32, tag="accS")
            accD = cpool.tile([128, mt, NNT], F32, tag="accD")

            lnc = cpool.tile([128, 1], F32, tag="lnc")
            nc.vector.memset(
                lnc[:, :], float(math.log(1.0 - math.exp(-S_SCALE * MARGIN))))
            csm = cpool.tile([128, 1], F32, tag="csm")
            nc.vector.memset(csm[:, :], float(S_SCALE * MARGIN))

            # main loop: fp8 DoubleRow matmuls into per-engine psum tiles
            # (psS: ScalarE table Exp w/ free accum row-sum; psD: DVE
            # Schraudolph exp2 via i32 affine + bitcast + reduce).
            NA = 5  # ntiles in collective phase A (classes [0, 9216))
            rr = cpool.tile([128, 2 * mt], F32, tag="rr")
            Bs = cpool.tile([128, 2], F32, tag="Bs")
            denA = cpool.tile([128, mt], F32, tag="denA")
            for ni, (n0, nw, scw) in enumerate(NTL):
                wt = wts[ni]
                dvw = nw - scw
                # chunk so no matmul write crosses a PSUM bank boundary
                # (psS/psD tiles are bank-aligned; scw itself need not
                # be a multiple of 512)
                nsub = [(j0, min(MMW, scw - j0)) for j0 in range(0, scw, MMW)]
                nsub += [(scw + j0, min(MMW, dvw - j0))
                         for j0 in range(0, dvw, MMW)]
                for m in range(mt):
                    psS = ppool.tile([128, 1536], F32, tag="psS", name="psS")
                    psD = ppool.tile([128, 512], F32, tag="psD", name="psD")
                    for k2 in range(kt2):
                        for j0, jw in nsub:
                            if j0 < scw:
                                dst = psS[:, j0 : j0 + jw]
                            else:
                                dst = psD[:, j0 - scw : j0 - scw + jw]
                            nc.tensor.matmul(
                                out=dst,
                                lhsT=xts[:, k2, :, m * 128 : (m + 1) * 128],
                                rhs=wt[:, k2, :, j0 : j0 + jw],
                                start=(k2 == 0), stop=(k2 == kt2 - 1),
                                perf_mode=mybir.MatmulPerfMode.DoubleRow)
                    # ScalarE share: exp -> bf16 with free row-sum accum
                    et = epool.tile([128, 1536], BF16, tag="et", name="et")
                    nc.scalar.activation(
                        et[:, :scw], psS[:, :scw], AF.Exp, bias=0.0, scale=KEXP,
                        accum_out=accS[:, m, ni : ni + 1])
                    # DVE share: Schraudolph exp2 (i16 convert + bf16
                    # bitcast; the reduce runs in DVE 2x mode)
                    ti = ipool.tile([128, 512], I16, tag="ti", name="ti")
                    nc.vector.tensor_scalar(
                        out=ti[:, :dvw], in0=psD[:, :dvw],
                        scalar1=float(A_SCH), scalar2=float(B_SCH),
                        op0=OP.mult, op1=OP.add)
                    nc.vector.tensor_reduce(
                        out=accD[:, m, ni : ni + 1],
                        in_=ti[:, :dvw].bitcast(BF16), axis=AX.X, op=OP.add)
                if ni == NA - 1:
                    # phase A: expsum partials for classes [0, 9216) + all
                    # target-logit partials; AllReduce overlaps ni 5-6.
                    ccsbA = cpool.tile([128, 2 * mt], F32, tag="ccsbA")
                    tmpA = cpool.tile([128, mt], F32, tag="tmpA")
                    nc.vector.tensor_reduce(
                        out=ccsbA[:, 0:mt], in_=accS[:, :, 0:NA],
                        axis=AX.X, op=OP.add)
                    nc.vector.tensor_reduce(
                        out=tmpA[:, :], in_=accD[:, :, 0:NA], axis=AX.X, op=OP.add)
                    nc.vector.tensor_tensor(
                        out=ccsbA[:, 0:mt], in0=ccsbA[:, 0:mt], in1=tmpA[:, :],
                        op=OP.add)
                    nc.vector.tensor_copy(out=ccsbA[:, mt : 2 * mt], in_=tz[:, :])
                    nc.sync.dma_start(ccA_in[:, :], ccsbA[:, :])
                    nc.gpsimd.collective_compute(
                        "AllReduce", OP.add,
                        replica_groups=[list(range(cores))],
                        ins=[ccA_in.ap().opt()], outs=[ccA_out.ap().opt()])
                if ni == NA:
                    # phase-A result readback (sync queue; waits on the
                    # collective, but nothing downstream consumes it
                    # until the tail, so no engine queue blocks on it)
                    nc.sync.dma_start(rr[:, :], ccA_out[:, :])

            # phase B: expsum partials for classes [9216, 12544)
            ccsbB = cpool.tile([128, mt], F32, tag="ccsbB")
            tmpB = cpool.tile([128, mt], F32, tag="tmpB")
            nc.vector.tensor_reduce(
                out=ccsbB[:, 0:mt], in_=accS[:, :, NA:NNT], axis=AX.X, op=OP.add)
            nc.vector.tensor_reduce(
                out=tmpB[:, :], in_=accD[:, :, NA:NNT], axis=AX.X, op=OP.add)
            nc.vector.tensor_tensor(
                out=ccsbB[:, 0:mt], in0=ccsbB[:, 0:mt], in1=tmpB[:, :], op=OP.add)
            nc.sync.dma_start(ccB_in[:, :], ccsbB[:, :])
            nc.gpsimd.collective_compute(
                "AllReduce", OP.add, replica_groups=[list(range(cores))],
                ins=[ccB_in.ap().opt()], outs=[ccB_out.ap().opt()])

            # phase-A epilogue (emitted at the tail so it can't
            # head-of-line block the Scalar/Vector queues behind the
            # collective): B = s*t_tot; Bs0 = sum(B);
            # Es = exp(B + ln(1-e^{-sm})); denA = expsumA - Es - pads
            B = cpool.tile([128, mt], F32, tag="B")
            nc.vector.tensor_scalar(
                out=B[:, :], in0=rr[:, mt : 2 * mt], scalar1=S_SCALE,
                scalar2=None, op0=OP.mult, op1=OP.add, accum_out=Bs[:, 0:1])
            Es = cpool.tile([128, mt], F32, tag="Es")
            nc.scalar.activation(
                Es[:, :], B[:, :], AF.Exp, bias=lnc[:, :1], scale=1.0)
            nc.vector.tensor_tensor(
                out=denA[:, :], in0=rr[:, 0:mt], in1=Es[:, :], op=OP.subtract)
            nc.vector.tensor_scalar(
                out=denA[:, :], in0=denA[:, :], scalar1=float(C_PAD_TOTAL),
                scalar2=None, op0=OP.subtract)

            rrB = cpool.tile([128, mt], F32, tag="rrB")
            nc.sync.dma_start(rrB[:, :], ccB_out[:, :])

            # exposed tail: den = denA + expsumB; loss = s*m - (sum(B) -
            # sum(ln den))/n
            den = cpool.tile([128, mt], F32, tag="den")
            nc.vector.tensor_tensor(
                out=den[:, :], in0=denA[:, :], in1=rrB[:, :], op=OP.add)
            lden = cpool.tile([128, mt], F32, tag="lden")
            nc.scalar.activation(
                lden[:, :], den[:, :], AF.Ln, accum_out=Bs[:, 1:2])
            diff = cpool.tile([128, 1], F32, tag="diff")
            nc.vector.tensor_tensor(
                out=diff[:, :], in0=Bs[:, 0:1], in1=Bs[:, 1:2], op=OP.subtract)
            zs = cpool.tile([128, 1], F32, tag="zs")
            nc.gpsimd.partition_all_reduce(zs[:, :], diff[:, :], 128, ReduceOp.add)
            res = cpool.tile([1, 1], F32, tag="res")
            nc.scalar.activation(
                res[:, :], zs[:1, :], AF.Identity,
                bias=csm[:1, :], scale=-1.0 / n)
            nc.sync.dma_start(out_d[:, :], res[:, :])
    nc.compile()
    return nc


def shard_inputs(x, labels, W, n=N, d=D, csh=CSH, cores=CORES):
    x32 = np.ascontiguousarray(np.asarray(x), dtype=np.float32)
    xn = x32 / np.sqrt((x32 * x32).sum(axis=1, keepdims=True))
    # [ki, k2, ko, n] fp8 DoubleRow layout: d = k2*256 + ko*128 + ki
    xq = (xn.T * XSCALE).reshape(2, 2, 128, n).transpose(2, 0, 1, 3)
    xq = np.ascontiguousarray(np.clip(xq, -240, 240)).astype(ml_dtypes.float8_e4m3)
    # [p, m, d] bf16 rows
    xnb = np.ascontiguousarray(
        xn.reshape(n // 128, 128, d).transpose(1, 0, 2)).astype(ml_dtypes.bfloat16)
    lab = np.asarray(labels).astype(np.int64).reshape(n)
    in_maps = []
    for r in range(cores):
        Wc = np.ascontiguousarray(np.asarray(W)[r * csh : (r + 1) * csh], dtype=np.float32)
        Wp = np.zeros((CSHP, d), np.float32)
        Wp[:csh] = Wc
        wq = (Wp.T * WSCALE).reshape(2, 2, 128, CSHP).transpose(2, 0, 1, 3)
        wq = np.ascontiguousarray(np.clip(wq, -240, 240)).astype(ml_dtypes.float8_e4m3)
        loc = lab - r * csh
        off = np.where((loc >= 0) & (loc < csh), loc, csh).astype(np.int32)
        off = np.ascontiguousarray(off.reshape(n // 128, 128).T)
        wn = np.vstack([Wc, np.zeros((1, d), np.float32)]).astype(ml_dtypes.bfloat16)
        in_maps.append({
            "xq": xq, "xn": xnb, "wq": wq,
            "wn": np.ascontiguousarray(wn), "off": off,
        })
    return in_maps


_CACHE = {}


def kernel(x, labels, W, **run_kwargs):
    if "nc" not in _CACHE:
        _CACHE["nc"] = build()
    nc = _CACHE["nc"]
    in_maps = shard_inputs(x, labels, W)
    res = run_bass_kernel_spmd(nc, in_maps, core_ids=list(range(CORES)), **run_kwargs)
    out = np.asarray(res.results[0]["out"], dtype=np.float32).reshape(())
    if run_kwargs:
        return out, res
    return out
